# revision 8
# baseline (speedup 1.0000x reference)
"""Trainium2 Bass kernel for causal multi-head attention (dense transformer block).

Problem: x[2,2048,1024] -> qkv proj -> 16-head causal attention (scale 1/sqrt(1024))
         -> out proj.  8 NeuronCores.

Sharding: core c handles batch b=c//4 and head-group r=c%4 (heads 4r..4r+3).
  - qkv weights column-sharded by head group (q/k/v slices of 256 cols each);
    x is passed to each core pre-transposed (dm-major) so the kernel stages
    x^T with plain cast-DMAs and no on-device transposes.
  - attention computed fully on-core in a transposed layout:
      S^T[k,q] = K^T-chunk (stationary) x Q^T (moving) on the PE
      P = exp(S/32); causal masking is done pre-exp by accumulating a
      -1e9 strict-upper-triangular matrix into the diagonal S block on the
      PE (mask matmul against an identity moving tile), so exp produces
      exact zeros below the diagonal and no post-exp masking is needed.
      The denominator comes from appending a ones-column to V so that
      O^T = [V|1]^T P gives sums in the last row.
  - q is processed in four 512-column passes; the qkv projection for token
    block p is emitted just before attention pass p, so weight/x staging and
    the out-proj of previous passes hide under attention compute.
  - attnV matmuls stream only the causally-valid P columns (al-trimmed
    partial-region PSUM accumulation), eliminating zero-fill memsets.
  - scores are emitted two jobs ahead (3 S-tile PSUM slots) so the PE never
    parks on the exp of the current job.
  - AllGather (bf16, groups of 4 cores sharing a batch) assembles all heads'
    outputs feature-major; out-proj is column-sharded with an all-gathered
    feature dim; biases are applied via rank-1 (K=1) matmul accumulation.

kernel(**inputs) takes the FULL fp32 inputs and returns the FULL output.
"""

import sys

sys.path.insert(0, "/opt/trn_rl_repo")

import numpy as np

import concourse.bass as bass
import concourse.bacc as bacc
import concourse.mybir as mybir
import concourse.tile as tile
from concourse.bass import ds, ts
from concourse.bass_utils import run_bass_kernel_spmd
from concourse.masks import make_upper_triangular

F32 = mybir.dt.float32
BF16 = mybir.dt.bfloat16

# ---------------------------------------------------------------- dims
BS, L, DM, H = 2, 2048, 1024, 16
HD = 64                      # head dim
NCORES = 8
GRP = 4                      # cores per batch group (head-parallel)
HLOC = H // GRP              # heads per core = 4
FLOC = HLOC * HD             # local features = 256
SCALE = 1.0 / float(np.sqrt(DM))
REPLICA_GROUPS = [[0, 1, 2, 3], [4, 5, 6, 7]]


class Cfg:
    """Geometry (parametrized so a small config can be tested quickly)."""

    def __init__(self, L=L, DM=DM, hloc=HLOC, hd=HD, npass=4):
        self.L, self.DM, self.HLOC, self.HD, self.NPASS = L, DM, hloc, hd, npass
        self.FLOC = hloc * hd
        self.NT = L // 128           # 128-token tiles
        self.NB = L // 512           # 512-token blocks
        self.NDM = DM // 128         # dmodel chunks
        self.PW = L // npass         # pass width (q columns per pass)
        self.NFT = self.FLOC // 128  # feature tiles for Q^T/K^T (2)
        self.scale = 1.0 / float(np.sqrt(DM))
        assert self.PW == 512 and self.FLOC % 128 == 0


def build_body(nc, cfg, x, wqkv, bq, bk, bv, wo, bo, out, groups):
    """Emit the per-core program (Tile framework) for one iteration.

    `x` here is the PRE-TRANSPOSED input: [DM, L] fp32 (dm-major).
    """
    NT, NB, NDM, PW, NFT = cfg.NT, cfg.NB, cfg.NDM, cfg.PW, cfg.NFT
    HLOCc, HDc, FLOCc = cfg.HLOC, cfg.HD, cfg.FLOC
    Lc, DMc = cfg.L, cfg.DM
    NPASS = cfg.NPASS
    tc = nc.tc

    with tc.tile_pool(name="const", bufs=1) as constp, \
         tc.tile_pool(name="persist", bufs=1) as pp:
        # ---------------- persistent SBUF tensors
        xT = pp.tile([128, NDM, Lc], BF16)                 # x^T  (dm-major)
        wqkvb = pp.tile([128, NDM, 3 * FLOCc], BF16)       # [wq|wk|wv] packed
        wqb = wqkvb[:, :, 0:FLOCc]
        wkb = wqkvb[:, :, FLOCc : 2 * FLOCc]
        wvb = wqkvb[:, :, 2 * FLOCc : 3 * FLOCc]
        wob = pp.tile([128, NDM, FLOCc], BF16)
        QT = pp.tile([128, NFT, Lc], BF16)                 # Q^T feature-major
        KT = pp.tile([128, NFT, Lc], BF16)
        Vb = pp.tile([128, NT, HLOCc * (HDc + 1)], BF16)   # [V | ones] per token tile
        OTs = pp.tile([128, NFT, Lc], BF16)                # attention out^T (feature-major)

        # ---------------- PSUM pools for the whole kernel
        # bank budget: stile [128,512]x4 = 4 (pool S), otile [65,512]x2 +
        # work [128,512]x2 = 4 (pool W)  ->  8 banks. Opened before staging,
        # so no pool-boundary barrier ever lands on the critical path.
        # 4 stile slots let scores run three jobs ahead and let the next
        # block's qkv matmuls start without waiting for trailing exps.
        psum_cm = tc.tile_pool(name="psumS", bufs=4, space="PSUM")
        psum = psum_cm.__enter__()
        psum2_cm = tc.tile_pool(name="psumW", bufs=2, space="PSUM")
        psum2 = psum2_cm.__enter__()

        # PE warmup: a few junk matmuls at the head so the p-state ramp
        # happens on dead time (the DMA-bound front), not on the first real
        # matmuls.
        NWARM = 8
        wsrc_t = pp.tile([128, 512], BF16, name="wsrc_t")
        nc.vector.memset(wsrc_t, 0.25)
        wps = psum.tile([128, 512], F32, tag="stile", name="wps")
        for r in range(NWARM):
            nc.tensor.matmul(wps, wsrc_t[:, 0:128], wsrc_t,
                             start=(r == 0), stop=(r == NWARM - 1))
        wout_t = pp.tile([128, 512], F32, name="wout_t")
        nc.vector.tensor_copy(wout_t, wps)

        # ---------------- constants
        maskM = constp.tile([128, 128], BF16)   # strict-upper -1e9 (q-row, k-col)
        ident = constp.tile([128, 128], BF16)   # identity (mask matmul moving tile)
        ones_r = constp.tile([1, 128], BF16)
        bq_f = constp.tile([128, NFT], F32)
        bk_f = constp.tile([128, NFT], F32)
        bvb = constp.tile([1, FLOCc], BF16)
        bob = constp.tile([1, FLOCc], BF16)

        def emit_consts_sync():
            # biases go over HWDGE (f32) + tiny DVE casts -- keeps the serial
            # Pool SWDGE queue free for the big weight/x cast-loads
            nc.sync.dma_start(bq_f, bq.rearrange("(f p) -> p f", p=128))
            nc.sync.dma_start(bk_f, bk.rearrange("(f p) -> p f", p=128))
            bv_st = constp.tile([1, 2 * FLOCc], F32, name="bv_st")
            nc.sync.dma_start(bv_st[:, 0:FLOCc], bv.rearrange("(a b) -> a b", a=1))
            nc.sync.dma_start(bv_st[:, FLOCc : 2 * FLOCc], bo.rearrange("(a b) -> a b", a=1))
            nc.vector.memset(ones_r, 1.0)
            nc.vector.tensor_copy(bvb, bv_st[:, 0:FLOCc])
            nc.vector.tensor_copy(bob, bv_st[:, FLOCc : 2 * FLOCc])
            # ones columns of Vb
            nc.vector.memset(
                Vb.rearrange("p t (h u) -> p t h u", u=HDc + 1)[:, :, :, HDc : HDc + 1], 1.0
            )

        def emit_consts_pool():
            # gpsimd-built mask constants: emitted AFTER the staging cast-DMAs
            # are queued so they don't delay the Pool SWDGE path.
            make_upper_triangular(nc, maskM, val=-1e9, diag=False)
            # identity: ones on/above diag, then keep only at-or-below diag
            make_upper_triangular(nc, ident, val=1.0, diag=True)
            nc.gpsimd.affine_select(
                out=ident, in_=ident,
                compare_op=mybir.AluOpType.is_ge,
                fill=0.0, base=0,
                pattern=[[-1, 128]], channel_multiplier=1,
            )

        # ---------------- weight + x staging
        # x arrives pre-transposed [DM, L]: each 512-token block loads with a
        # single fp32->bf16 cast DMA straight into the chunked xT layout.
        # wqkv is loaded in two dm-chunk halves so the first qkv matmuls
        # (which consume dm-chunks in order) start as soon as x-block0 and
        # the first half have landed.
        xTv = x.rearrange("(c p) t -> p c t", p=128)
        wv_ = wqkv.rearrange("(c p) f -> p c f", p=128)

        def stage_xblock(b):
            nc.gpsimd.dma_start(xT[:, :, ts(b, 512)], xTv[:, :, ts(b, 512)])

        stage_xblock(0)
        nc.gpsimd.dma_start(wqkvb[:, 0 : NDM // 2, :], wv_[:, 0 : NDM // 2, :])
        emit_consts_sync()
        nc.gpsimd.dma_start(wqkvb[:, NDM // 2 : NDM, :], wv_[:, NDM // 2 : NDM, :])
        for b in range(1, NB):
            stage_xblock(b)
        nc.gpsimd.dma_start(wob, wo.rearrange("(c p) f -> p c f", p=128))
        emit_consts_pool()

        # ---------------- qkv projection for one 512-token block
        def emit_qkv(tb):
            for ft in range(NFT):
                qs_ = psum.tile([128, 512], F32, tag="stile", name="qs")
                for c in range(NDM):
                    nc.tensor.matmul(
                        qs_, wqb[:, c, ts(ft, 128)], xT[:, c, ts(tb, 512)],
                        start=(c == 0), stop=(c == NDM - 1),
                    )
                nc.scalar.activation(QT[:, ft, ts(tb, 512)], qs_,
                                     mybir.ActivationFunctionType.Identity,
                                     bias=bq_f[:, ft : ft + 1])
                ks_ = psum.tile([128, 512], F32, tag="stile", name="ks")
                for c in range(NDM):
                    nc.tensor.matmul(
                        ks_, wkb[:, c, ts(ft, 128)], xT[:, c, ts(tb, 512)],
                        start=(c == 0), stop=(c == NDM - 1),
                    )
                nc.scalar.activation(KT[:, ft, ts(tb, 512)], ks_,
                                     mybir.ActivationFunctionType.Identity,
                                     bias=bk_f[:, ft : ft + 1])
            for tt in range(tb * 4, tb * 4 + 4):
                psv_full = psum2.tile([128, 512], F32, tag="work", name="psv_full")
                psv = psv_full[:, 0:FLOCc]
                for c in range(NDM):
                    nc.tensor.matmul(
                        psv, xT[:, c, ts(tt, 128)], wvb[:, c, :],
                        start=(c == 0), stop=False,
                    )
                nc.tensor.matmul(psv, ones_r, bvb, start=False, stop=True)
                nc.scalar.copy(
                    Vb[:, tt, :].rearrange("p (h u) -> p h u", u=HDc + 1)[:, :, 0:HDc],
                    psv.rearrange("p (h d) -> p h d", d=HDc),
                )

        # ---------------- attention + allgather + out projection
        with tc.tile_pool(name="pbuf", bufs=6) as pbp, \
             tc.tile_pool(name="nrm", bufs=6) as nrm, \
             tc.tile_pool(name="of", bufs=3) as ofp, \
             tc.tile_pool(name="osb", bufs=3) as osbp, \
             tc.tile_pool(name="dram", bufs=2, space="DRAM") as dramp:

            # ---- allgather + out-projection for one 512-token pass.
            # Emission is deferred into the NEXT pass's job stream: PE.SEQ is
            # in-order, and out-proj parked on the AllGather would otherwise
            # stall the next pass's scores.
            def emit_agproj(p):
                q0 = p * PW
                ag_in = dramp.tile([NFT * 128, 512], BF16, tag="agin", name="ag_in")
                # NOTE: Shared-output collectives need >4 cores/group; with
                # 4-core groups the output must be a Local scratch tensor.
                ag_out = dramp.tile([GRP * NFT * 128, 512], BF16, tag="agout", name="ag_out")
                for t in range(NFT):
                    nc.sync.dma_start(ag_in[ts(t, 128), :], OTs[:, t, ds(q0, 512)])
                nc.gpsimd.collective_compute(
                    "AllGather",
                    mybir.AluOpType.bypass,
                    ins=[ag_in.opt()],
                    outs=[ag_out.opt()],
                    replica_groups=groups,
                )
                OF = ofp.tile([128, NDM, 512], BF16, tag="of", name="OF")
                # paired per-chunk loads: the first out-proj matmul starts
                # after 256KB instead of the full 1MB gathered-feature
                # transfer, at half the HWDGE slots of per-chunk loads
                agv = ag_out.rearrange("(c p) q -> p c q", p=128)
                for c2 in range(NDM // 2):
                    nc.sync.dma_start(OF[:, 2 * c2 : 2 * c2 + 2, :],
                                      agv[:, 2 * c2 : 2 * c2 + 2, :])
                osb = osbp.tile([128, 4, FLOCc], F32, tag="osb", name="osb")
                outv = out[ds(q0, 512), :].rearrange("(t p) f -> p t f", p=128)
                for ttl in range(4):
                    pout_full = psum2.tile([128, 512], F32, tag="work", name="pout_full")
                    pout = pout_full[:, 0:FLOCc]
                    for c in range(NDM):
                        nc.tensor.matmul(
                            pout, OF[:, c, ts(ttl, 128)], wob[:, c, :],
                            start=(c == 0), stop=False,
                        )
                    nc.tensor.matmul(pout, ones_r, bob, start=False, stop=True)
                    nc.vector.tensor_copy(osb[:, ttl, :], pout)
                    if ttl == 1:
                        nc.sync.dma_start(outv[:, 0:2, :], osb[:, 0:2, :])
                nc.sync.dma_start(outv[:, 2:4, :], osb[:, 2:4, :])

            # ---- half-feature AllGather for the FINAL pass: heads {0,1}
            # (= OTs feature tile 0) gather while heads {2,3} still compute,
            # so only the second half-gather sits on the kernel tail.
            # Gathered chunk r of half ft maps to wob chunk 2r+ft.
            def emit_ag_half(p, fth):
                q0 = p * PW
                ag_in_h = dramp.tile([128, 512], BF16, tag="aginh", name="ag_in_h")
                ag_out_h = dramp.tile([GRP * 128, 512], BF16, tag="agouth", name="ag_out_h")
                nc.sync.dma_start(ag_in_h, OTs[:, fth, ds(q0, 512)])
                nc.gpsimd.collective_compute(
                    "AllGather",
                    mybir.AluOpType.bypass,
                    ins=[ag_in_h.opt()],
                    outs=[ag_out_h.opt()],
                    replica_groups=groups,
                )
                OFh = ofp.tile([128, GRP, 512], BF16, tag="ofh", name="OFh")
                agvh = ag_out_h.rearrange("(c p) q -> p c q", p=128)
                for c2 in range(GRP // 2):
                    nc.sync.dma_start(OFh[:, 2 * c2 : 2 * c2 + 2, :],
                                      agvh[:, 2 * c2 : 2 * c2 + 2, :])
                return OFh

            def emit_outproj_split(p, OFA, OFB):
                q0 = p * PW
                osb = osbp.tile([128, 4, FLOCc], F32, tag="osb", name="osb")
                outv = out[ds(q0, 512), :].rearrange("(t p) f -> p t f", p=128)
                for ttl in range(4):
                    pout_full = psum2.tile([128, 512], F32, tag="work", name="pout_full")
                    pout = pout_full[:, 0:FLOCc]
                    for r in range(GRP):
                        nc.tensor.matmul(
                            pout, OFA[:, r, ts(ttl, 128)], wob[:, 2 * r, :],
                            start=(r == 0), stop=False,
                        )
                    for r in range(GRP):
                        nc.tensor.matmul(
                            pout, OFB[:, r, ts(ttl, 128)], wob[:, 2 * r + 1, :],
                            start=False, stop=False,
                        )
                    nc.tensor.matmul(pout, ones_r, bob, start=False, stop=True)
                    nc.vector.tensor_copy(osb[:, ttl, :], pout)
                    nc.sync.dma_start(outv[:, ttl, :], osb[:, ttl, :])

            for p in range(NPASS):
                emit_qkv(p)
                ilast = 4 * p + 3
                qs0 = p * PW

                def emit_scores(h, i):
                    """Scores for (head h, key tile i), causally trimmed, with
                    the -1e9 mask matmul folded into the diagonal block's
                    accumulation group."""
                    hf, hp = h // 2, h % 2
                    al = max(0, 128 * i - qs0)
                    is_diag = 128 * i >= qs0
                    S = psum.tile([128, PW], F32, tag="stile", name="S")
                    nc.tensor.matmul(
                        S[:, ds(al, PW - al)],
                        KT[64 * hp : 64 * hp + 64, hf, ts(i, 128)],
                        QT[64 * hp : 64 * hp + 64, hf, ds(qs0 + al, PW - al)],
                        start=True, stop=not is_diag,
                        skip_group_check=is_diag,
                    )
                    if is_diag:
                        # S[k, q] += -1e9 where k > q on the diagonal
                        # 128-block: lhsT = strict-upper(-1e9) so that
                        # (maskM^T @ I)[k, q] = maskM[q, k]
                        nc.tensor.matmul(
                            S[:, ds(al, 128)],
                            maskM, ident,
                            start=False, stop=True,
                            skip_group_check=True,
                        )
                    return S

                # one flat (h, i) stream with scores emitted TWO steps ahead:
                # PE.SEQ is in-order, so S(next) instructions must be issued
                # before attnV(cur) parks the queue on exp(cur).
                jobs = [(h, i) for h in range(HLOCc) for i in range(ilast + 1)]
                sp1 = len(jobs) // 4
                sp_half = 2 * (ilast + 1) + 1  # just after heads {0,1} finish
                OFA = None
                po_all = {}
                Sq = [emit_scores(*jobs[k]) for k in range(min(3, len(jobs)))]
                for idx, (h, i) in enumerate(jobs):
                    hf, hp = h // 2, h % 2
                    if p > 0 and idx == sp1:
                        emit_agproj(p - 1)
                    if p == NPASS - 1 and idx == sp_half:
                        OFA = emit_ag_half(p, 0)
                    S = Sq.pop(0)
                    if idx + 3 < len(jobs):
                        Sq.append(emit_scores(*jobs[idx + 3]))
                    if i == 0:
                        po_all[h] = psum2.tile([HDc + 1, 512], F32, tag="otile", name="po")
                    po = po_all[h]
                    al = max(0, 128 * i - qs0)
                    P = pbp.tile([128, PW], BF16, tag="ptile", name="P")
                    nc.scalar.activation(
                        P[:, ds(al, PW - al)],
                        S[:, ds(al, PW - al)],
                        mybir.ActivationFunctionType.Exp,
                        scale=float(cfg.scale),
                    )
                    nc.tensor.matmul(
                        po[:, ds(al, PW - al)],
                        Vb[:, i, ds((HDc + 1) * h, HDc + 1)],
                        P[:, ds(al, PW - al)],
                        start=(i == 0), stop=(i == ilast),
                        skip_group_check=(al > 0 or i != ilast),
                    )
                    if i == ilast:
                        # the copy exists to free the PSUM accumulator for the
                        # next head; the last head of the last pass normalizes
                        # straight from PSUM (shorter end-of-kernel chain)
                        if p == NPASS - 1 and h == HLOCc - 1:
                            osrc = po
                        else:
                            osrc = nrm.tile([HDc + 1, 512], F32, tag="osnap", name="osnap")
                            nc.vector.tensor_copy(osrc, po)
                        rec = nrm.tile([1, 512], F32, tag="rec", name="rec")
                        nc.vector.reciprocal(rec, osrc[HDc : HDc + 1, :])
                        rb = nrm.tile([64, 512], F32, tag="rb", name="rb")
                        nc.gpsimd.partition_broadcast(rb, rec)
                        nc.vector.tensor_mul(
                            OTs[64 * hp : 64 * hp + 64, hf, ds(qs0, 512)],
                            osrc[0:HDc, :],
                            rb,
                        )
                if p == NPASS - 1:
                    OFB = emit_ag_half(p, 1)
                    emit_outproj_split(p, OFA, OFB)
        psum2_cm.__exit__(None, None, None)
        psum_cm.__exit__(None, None, None)


def make_program(cfg=None, groups=None, unroll=1):
    cfg = cfg or Cfg()
    groups = groups or REPLICA_GROUPS
    nc = bacc.Bacc("TRN2", target_bir_lowering=False, debug=False, num_devices=NCORES)
    x = nc.dram_tensor("x", [cfg.DM, cfg.L], F32, kind="ExternalInput").ap()
    wqkv = nc.dram_tensor("wqkv", [cfg.DM, 3 * cfg.FLOC], F32, kind="ExternalInput").ap()
    bq = nc.dram_tensor("bq", [cfg.FLOC], F32, kind="ExternalInput").ap()
    bk = nc.dram_tensor("bk", [cfg.FLOC], F32, kind="ExternalInput").ap()
    bv = nc.dram_tensor("bv", [cfg.FLOC], F32, kind="ExternalInput").ap()
    wo = nc.dram_tensor("wo", [cfg.DM, cfg.FLOC], F32, kind="ExternalInput").ap()
    bo = nc.dram_tensor("bo", [cfg.FLOC], F32, kind="ExternalInput").ap()
    out = nc.dram_tensor("out", [cfg.L, cfg.FLOC], F32, kind="ExternalOutput").ap()
    with tile.TileContext(nc) as tc:
        nc.tc = tc
        for _ in range(unroll):
            build_body(nc, cfg, x, wqkv, bq, bk, bv, wo, bo, out, groups)
    nc.compile()
    return nc


def shard_inputs(x, w_qkv, b_qkv, w_out, b_out, cfg=None):
    """Full inputs -> list of 8 per-core input dicts (x pre-transposed)."""
    cfg = cfg or Cfg()
    FL = cfg.FLOC
    DMF = cfg.DM
    in_maps = []
    xt = [np.ascontiguousarray(x[b].T) for b in range(BS)]
    for c in range(NCORES):
        b, r = divmod(c, GRP)
        q0 = r * FL
        in_maps.append({
            "x": xt[b],
            "wqkv": np.ascontiguousarray(np.concatenate([
                w_qkv[:, q0 : q0 + FL],
                w_qkv[:, DMF + q0 : DMF + q0 + FL],
                w_qkv[:, 2 * DMF + q0 : 2 * DMF + q0 + FL],
            ], axis=1)),
            "bq": np.ascontiguousarray(b_qkv[q0 : q0 + FL]),
            "bk": np.ascontiguousarray(b_qkv[DMF + q0 : DMF + q0 + FL]),
            "bv": np.ascontiguousarray(b_qkv[2 * DMF + q0 : 2 * DMF + q0 + FL]),
            "wo": np.ascontiguousarray(w_out[:, q0 : q0 + FL]),
            "bo": np.ascontiguousarray(b_out[q0 : q0 + FL]),
        })
    return in_maps


def gather_output(results, cfg=None):
    cfg = cfg or Cfg()
    FL = cfg.FLOC
    out = np.empty((BS, cfg.L, cfg.DM), np.float32)
    for c in range(NCORES):
        b, r = divmod(c, GRP)
        out[b, :, r * FL : (r + 1) * FL] = results[c]["out"]
    return out


_PROGRAM = None


def _get_program():
    global _PROGRAM
    if _PROGRAM is None:
        _PROGRAM = make_program()
    return _PROGRAM


def kernel(x, w_qkv, b_qkv, w_out, b_out):
    x = np.asarray(x, np.float32)
    w_qkv = np.asarray(w_qkv, np.float32)
    b_qkv = np.asarray(b_qkv, np.float32)
    w_out = np.asarray(w_out, np.float32)
    b_out = np.asarray(b_out, np.float32)
    nc = _get_program()
    in_maps = shard_inputs(x, w_qkv, b_qkv, w_out, b_out)
    res = run_bass_kernel_spmd(nc, in_maps, list(range(NCORES)))
    return gather_output(res.results)


# revision 9
# speedup vs baseline: 1.0314x; 1.0314x over previous
"""Trainium2 Bass kernel for causal multi-head attention (dense transformer block).

Problem: x[2,2048,1024] -> qkv proj -> 16-head causal attention (scale 1/sqrt(1024))
         -> out proj.  8 NeuronCores.

Sharding: core c handles batch b=c//4 and head-group r=c%4 (heads 4r..4r+3).
  - qkv weights column-sharded by head group (q/k/v slices of 256 cols each);
    x is passed to each core pre-transposed (dm-major) so the kernel stages
    x^T with plain cast-DMAs and no on-device transposes.
  - attention computed fully on-core in a transposed layout:
      S^T[k,q] = K^T-chunk (stationary) x Q^T (moving) on the PE
      P = exp(S/32); causal masking is done pre-exp by accumulating a
      -1e9 strict-upper-triangular matrix into the diagonal S block on the
      PE (mask matmul against an identity moving tile), so exp produces
      exact zeros below the diagonal and no post-exp masking is needed.
      The denominator comes from appending a ones-column to V so that
      O^T = [V|1]^T P gives sums in the last row.
  - q is processed in four 512-column passes; the qkv projection for token
    block p is emitted just before attention pass p, so weight/x staging and
    the out-proj of previous passes hide under attention compute.
  - attnV matmuls stream only the causally-valid P columns (al-trimmed
    partial-region PSUM accumulation), eliminating zero-fill memsets.
  - scores are emitted two jobs ahead (3 S-tile PSUM slots) so the PE never
    parks on the exp of the current job.
  - AllGather (bf16, groups of 4 cores sharing a batch) assembles all heads'
    outputs feature-major; out-proj is column-sharded with an all-gathered
    feature dim; biases are applied via rank-1 (K=1) matmul accumulation.

kernel(**inputs) takes the FULL fp32 inputs and returns the FULL output.
"""

import sys

sys.path.insert(0, "/opt/trn_rl_repo")

import numpy as np

import concourse.bass as bass
import concourse.bacc as bacc
import concourse.mybir as mybir
import concourse.tile as tile
from concourse.bass import ds, ts
from concourse.bass_utils import run_bass_kernel_spmd
from concourse.masks import make_upper_triangular

F32 = mybir.dt.float32
BF16 = mybir.dt.bfloat16

# ---------------------------------------------------------------- dims
BS, L, DM, H = 2, 2048, 1024, 16
HD = 64                      # head dim
NCORES = 8
GRP = 4                      # cores per batch group (head-parallel)
HLOC = H // GRP              # heads per core = 4
FLOC = HLOC * HD             # local features = 256
SCALE = 1.0 / float(np.sqrt(DM))
REPLICA_GROUPS = [[0, 1, 2, 3], [4, 5, 6, 7]]


class Cfg:
    """Geometry (parametrized so a small config can be tested quickly)."""

    def __init__(self, L=L, DM=DM, hloc=HLOC, hd=HD, npass=4):
        self.L, self.DM, self.HLOC, self.HD, self.NPASS = L, DM, hloc, hd, npass
        self.FLOC = hloc * hd
        self.NT = L // 128           # 128-token tiles
        self.NB = L // 512           # 512-token blocks
        self.NDM = DM // 128         # dmodel chunks
        self.PW = L // npass         # pass width (q columns per pass)
        self.NFT = self.FLOC // 128  # feature tiles for Q^T/K^T (2)
        self.scale = 1.0 / float(np.sqrt(DM))
        assert self.PW == 512 and self.FLOC % 128 == 0


def build_body(nc, cfg, x, wqkv, bq, bk, bv, wo, bo, out, groups):
    """Emit the per-core program (Tile framework) for one iteration.

    `x` here is the PRE-TRANSPOSED input: [DM, L] fp32 (dm-major).
    """
    NT, NB, NDM, PW, NFT = cfg.NT, cfg.NB, cfg.NDM, cfg.PW, cfg.NFT
    HLOCc, HDc, FLOCc = cfg.HLOC, cfg.HD, cfg.FLOC
    Lc, DMc = cfg.L, cfg.DM
    NPASS = cfg.NPASS
    tc = nc.tc

    with tc.tile_pool(name="const", bufs=1) as constp, \
         tc.tile_pool(name="persist", bufs=1) as pp:
        # ---------------- persistent SBUF tensors
        xT = pp.tile([128, NDM, Lc], BF16)                 # x^T  (dm-major)
        wqkvb = pp.tile([128, NDM, 3 * FLOCc], BF16)       # [wq|wk|wv] packed
        wqb = wqkvb[:, :, 0:FLOCc]
        wkb = wqkvb[:, :, FLOCc : 2 * FLOCc]
        wvb = wqkvb[:, :, 2 * FLOCc : 3 * FLOCc]
        wob = pp.tile([128, NDM, FLOCc], BF16)
        QT = pp.tile([128, NFT, Lc], BF16)                 # Q^T feature-major
        KT = pp.tile([128, NFT, Lc], BF16)
        Vb = pp.tile([128, NT, HLOCc * (HDc + 1)], BF16)   # [V | ones] per token tile
        OTs = pp.tile([128, NFT, Lc], BF16)                # attention out^T (feature-major)

        # ---------------- PSUM pools for the whole kernel
        # bank budget: stile [128,512]x4 = 4 (pool S), otile [65,512]x2 +
        # work [128,512]x2 = 4 (pool W)  ->  8 banks. Opened before staging,
        # so no pool-boundary barrier ever lands on the critical path.
        # 4 stile slots let scores run three jobs ahead and let the next
        # block's qkv matmuls start without waiting for trailing exps.
        psum_cm = tc.tile_pool(name="psumS", bufs=4, space="PSUM")
        psum = psum_cm.__enter__()
        psum2_cm = tc.tile_pool(name="psumW", bufs=2, space="PSUM")
        psum2 = psum2_cm.__enter__()

        # PE warmup: a few junk matmuls at the head so the p-state ramp
        # happens on dead time (the DMA-bound front), not on the first real
        # matmuls.
        NWARM = 12
        wsrc_t = pp.tile([128, 512], BF16, name="wsrc_t")
        nc.vector.memset(wsrc_t, 0.25)
        wps = psum.tile([128, 512], F32, tag="stile", name="wps")
        for r in range(NWARM):
            nc.tensor.matmul(wps, wsrc_t[:, 0:128], wsrc_t,
                             start=(r == 0), stop=(r == NWARM - 1))
        wout_t = pp.tile([128, 512], F32, name="wout_t")
        nc.vector.tensor_copy(wout_t, wps)

        # ---------------- constants
        maskM = constp.tile([128, 128], BF16)   # strict-upper -1e9 (q-row, k-col)
        ident = constp.tile([128, 128], BF16)   # identity (mask matmul moving tile)
        ones_r = constp.tile([1, 128], BF16)
        bq_f = constp.tile([128, NFT], F32)
        bk_f = constp.tile([128, NFT], F32)
        bvb = constp.tile([1, FLOCc], BF16)
        bob = constp.tile([1, FLOCc], BF16)

        def emit_consts_sync():
            # biases go over HWDGE (f32) + tiny DVE casts -- keeps the serial
            # Pool SWDGE queue free for the big weight/x cast-loads
            nc.sync.dma_start(bq_f, bq.rearrange("(f p) -> p f", p=128))
            nc.sync.dma_start(bk_f, bk.rearrange("(f p) -> p f", p=128))
            bv_st = constp.tile([1, 2 * FLOCc], F32, name="bv_st")
            nc.sync.dma_start(bv_st[:, 0:FLOCc], bv.rearrange("(a b) -> a b", a=1))
            nc.sync.dma_start(bv_st[:, FLOCc : 2 * FLOCc], bo.rearrange("(a b) -> a b", a=1))
            nc.vector.memset(ones_r, 1.0)
            nc.vector.tensor_copy(bvb, bv_st[:, 0:FLOCc])
            nc.vector.tensor_copy(bob, bv_st[:, FLOCc : 2 * FLOCc])
            # ones columns of Vb
            nc.vector.memset(
                Vb.rearrange("p t (h u) -> p t h u", u=HDc + 1)[:, :, :, HDc : HDc + 1], 1.0
            )

        def emit_consts_pool():
            # gpsimd-built mask constants: emitted AFTER the staging cast-DMAs
            # are queued so they don't delay the Pool SWDGE path.
            make_upper_triangular(nc, maskM, val=-1e9, diag=False)
            # identity: ones on/above diag, then keep only at-or-below diag
            make_upper_triangular(nc, ident, val=1.0, diag=True)
            nc.gpsimd.affine_select(
                out=ident, in_=ident,
                compare_op=mybir.AluOpType.is_ge,
                fill=0.0, base=0,
                pattern=[[-1, 128]], channel_multiplier=1,
            )

        # ---------------- weight + x staging
        # x arrives pre-transposed [DM, L]: each 512-token block loads with a
        # single fp32->bf16 cast DMA straight into the chunked xT layout.
        # wqkv is loaded in two dm-chunk halves so the first qkv matmuls
        # (which consume dm-chunks in order) start as soon as x-block0 and
        # the first half have landed.
        xTv = x.rearrange("(c p) t -> p c t", p=128)
        wv_ = wqkv.rearrange("(c p) f -> p c f", p=128)

        def stage_xblock(b):
            nc.gpsimd.dma_start(xT[:, :, ts(b, 512)], xTv[:, :, ts(b, 512)])

        nc.gpsimd.dma_start(wqkvb[:, 0 : NDM // 2, :], wv_[:, 0 : NDM // 2, :])
        stage_xblock(0)
        emit_consts_sync()
        nc.gpsimd.dma_start(wqkvb[:, NDM // 2 : NDM, :], wv_[:, NDM // 2 : NDM, :])
        for b in range(1, NB):
            stage_xblock(b)
        nc.gpsimd.dma_start(wob, wo.rearrange("(c p) f -> p c f", p=128))
        emit_consts_pool()

        # ---------------- qkv projection for one 512-token block
        def emit_qkv(tb):
            for ft in range(NFT):
                qs_ = psum.tile([128, 512], F32, tag="stile", name="qs")
                for c in range(NDM):
                    nc.tensor.matmul(
                        qs_, wqb[:, c, ts(ft, 128)], xT[:, c, ts(tb, 512)],
                        start=(c == 0), stop=(c == NDM - 1),
                    )
                nc.scalar.activation(QT[:, ft, ts(tb, 512)], qs_,
                                     mybir.ActivationFunctionType.Identity,
                                     bias=bq_f[:, ft : ft + 1])
                ks_ = psum.tile([128, 512], F32, tag="stile", name="ks")
                for c in range(NDM):
                    nc.tensor.matmul(
                        ks_, wkb[:, c, ts(ft, 128)], xT[:, c, ts(tb, 512)],
                        start=(c == 0), stop=(c == NDM - 1),
                    )
                nc.scalar.activation(KT[:, ft, ts(tb, 512)], ks_,
                                     mybir.ActivationFunctionType.Identity,
                                     bias=bk_f[:, ft : ft + 1])
            for tt in range(tb * 4, tb * 4 + 4):
                psv_full = psum2.tile([128, 512], F32, tag="work", name="psv_full")
                psv = psv_full[:, 0:FLOCc]
                for c in range(NDM):
                    nc.tensor.matmul(
                        psv, xT[:, c, ts(tt, 128)], wvb[:, c, :],
                        start=(c == 0), stop=False,
                    )
                nc.tensor.matmul(psv, ones_r, bvb, start=False, stop=True)
                nc.scalar.copy(
                    Vb[:, tt, :].rearrange("p (h u) -> p h u", u=HDc + 1)[:, :, 0:HDc],
                    psv.rearrange("p (h d) -> p h d", d=HDc),
                )

        # ---------------- attention + allgather + out projection
        with tc.tile_pool(name="pbuf", bufs=6) as pbp, \
             tc.tile_pool(name="nrm", bufs=6) as nrm, \
             tc.tile_pool(name="of", bufs=3) as ofp, \
             tc.tile_pool(name="osb", bufs=3) as osbp, \
             tc.tile_pool(name="dram", bufs=2, space="DRAM") as dramp:

            # ---- allgather + out-projection for one 512-token pass.
            # Emission is deferred into the NEXT pass's job stream: PE.SEQ is
            # in-order, and out-proj parked on the AllGather would otherwise
            # stall the next pass's scores.
            def emit_ag(p):
                """AllGather of pass p's OTs block + gathered-feature loads.
                Emitted right after the pass's jobs: everything here rides the
                DMA/Pool queues, so it cannot park the in-order PE queue."""
                q0 = p * PW
                ag_in = dramp.tile([NFT * 128, 512], BF16, tag="agin", name="ag_in")
                # NOTE: Shared-output collectives need >4 cores/group; with
                # 4-core groups the output must be a Local scratch tensor.
                ag_out = dramp.tile([GRP * NFT * 128, 512], BF16, tag="agout", name="ag_out")
                for t in range(NFT):
                    nc.sync.dma_start(ag_in[ts(t, 128), :], OTs[:, t, ds(q0, 512)])
                nc.gpsimd.collective_compute(
                    "AllGather",
                    mybir.AluOpType.bypass,
                    ins=[ag_in.opt()],
                    outs=[ag_out.opt()],
                    replica_groups=groups,
                )
                OF = ofp.tile([128, NDM, 512], BF16, tag="of", name="OF")
                # paired per-chunk loads at half the HWDGE slots
                agv = ag_out.rearrange("(c p) q -> p c q", p=128)
                for c2 in range(NDM // 2):
                    nc.sync.dma_start(OF[:, 2 * c2 : 2 * c2 + 2, :],
                                      agv[:, 2 * c2 : 2 * c2 + 2, :])
                return OF

            def emit_outproj(p, OF):
                q0 = p * PW
                osb = osbp.tile([128, 4, FLOCc], F32, tag="osb", name="osb")
                outv = out[ds(q0, 512), :].rearrange("(t p) f -> p t f", p=128)
                for ttl in range(4):
                    pout_full = psum2.tile([128, 512], F32, tag="work", name="pout_full")
                    pout = pout_full[:, 0:FLOCc]
                    for c in range(NDM):
                        nc.tensor.matmul(
                            pout, OF[:, c, ts(ttl, 128)], wob[:, c, :],
                            start=(c == 0), stop=False,
                        )
                    nc.tensor.matmul(pout, ones_r, bob, start=False, stop=True)
                    nc.vector.tensor_copy(osb[:, ttl, :], pout)
                    if ttl == 1:
                        nc.sync.dma_start(outv[:, 0:2, :], osb[:, 0:2, :])
                nc.sync.dma_start(outv[:, 2:4, :], osb[:, 2:4, :])

            # ---- half-feature AllGather for the FINAL pass: heads {0,1}
            # (= OTs feature tile 0) gather while heads {2,3} still compute,
            # so only the second half-gather sits on the kernel tail.
            # Gathered chunk r of half ft maps to wob chunk 2r+ft.
            def emit_ag_half(p, fth):
                q0 = p * PW
                ag_in_h = dramp.tile([128, 512], BF16, tag="aginh", name="ag_in_h")
                ag_out_h = dramp.tile([GRP * 128, 512], BF16, tag="agouth", name="ag_out_h")
                nc.sync.dma_start(ag_in_h, OTs[:, fth, ds(q0, 512)])
                nc.gpsimd.collective_compute(
                    "AllGather",
                    mybir.AluOpType.bypass,
                    ins=[ag_in_h.opt()],
                    outs=[ag_out_h.opt()],
                    replica_groups=groups,
                )
                OFh = ofp.tile([128, GRP, 512], BF16, tag="ofh", name="OFh")
                agvh = ag_out_h.rearrange("(c p) q -> p c q", p=128)
                for c2 in range(GRP // 2):
                    nc.sync.dma_start(OFh[:, 2 * c2 : 2 * c2 + 2, :],
                                      agvh[:, 2 * c2 : 2 * c2 + 2, :])
                return OFh

            def emit_outproj_split(p, OFA, OFB):
                q0 = p * PW
                osb = osbp.tile([128, 4, FLOCc], F32, tag="osb", name="osb")
                outv = out[ds(q0, 512), :].rearrange("(t p) f -> p t f", p=128)
                for ttl in range(4):
                    pout_full = psum2.tile([128, 512], F32, tag="work", name="pout_full")
                    pout = pout_full[:, 0:FLOCc]
                    for r in range(GRP):
                        nc.tensor.matmul(
                            pout, OFA[:, r, ts(ttl, 128)], wob[:, 2 * r, :],
                            start=(r == 0), stop=False,
                        )
                    for r in range(GRP):
                        nc.tensor.matmul(
                            pout, OFB[:, r, ts(ttl, 128)], wob[:, 2 * r + 1, :],
                            start=False, stop=False,
                        )
                    nc.tensor.matmul(pout, ones_r, bob, start=False, stop=True)
                    nc.vector.tensor_copy(osb[:, ttl, :], pout)
                    nc.sync.dma_start(outv[:, ttl, :], osb[:, ttl, :])

            for p in range(NPASS):
                emit_qkv(p)
                ilast = 4 * p + 3
                qs0 = p * PW

                def emit_scores(h, i):
                    """Scores for (head h, key tile i), causally trimmed, with
                    the -1e9 mask matmul folded into the diagonal block's
                    accumulation group."""
                    hf, hp = h // 2, h % 2
                    al = max(0, 128 * i - qs0)
                    is_diag = 128 * i >= qs0
                    S = psum.tile([128, PW], F32, tag="stile", name="S")
                    nc.tensor.matmul(
                        S[:, ds(al, PW - al)],
                        KT[64 * hp : 64 * hp + 64, hf, ts(i, 128)],
                        QT[64 * hp : 64 * hp + 64, hf, ds(qs0 + al, PW - al)],
                        start=True, stop=not is_diag,
                        skip_group_check=is_diag,
                    )
                    if is_diag:
                        # S[k, q] += -1e9 where k > q on the diagonal
                        # 128-block: lhsT = strict-upper(-1e9) so that
                        # (maskM^T @ I)[k, q] = maskM[q, k]
                        nc.tensor.matmul(
                            S[:, ds(al, 128)],
                            maskM, ident,
                            start=False, stop=True,
                            skip_group_check=True,
                        )
                    return S

                # one flat (h, i) stream with scores emitted TWO steps ahead:
                # PE.SEQ is in-order, so S(next) instructions must be issued
                # before attnV(cur) parks the queue on exp(cur).
                jobs = [(h, i) for h in range(HLOCc) for i in range(ilast + 1)]
                sp1 = len(jobs) // 4
                sp_half = 2 * (ilast + 1) + 1  # just after heads {0,1} finish
                OFA = None
                po_all = {}
                Sq = [emit_scores(*jobs[k]) for k in range(min(3, len(jobs)))]
                for idx, (h, i) in enumerate(jobs):
                    hf, hp = h // 2, h % 2
                    if p > 0 and idx == sp1:
                        emit_outproj(p - 1, OF_prev)
                    if p == NPASS - 1 and idx == sp_half:
                        OFA = emit_ag_half(p, 0)
                    S = Sq.pop(0)
                    if idx + 3 < len(jobs):
                        Sq.append(emit_scores(*jobs[idx + 3]))
                    if i == 0:
                        po_all[h] = psum2.tile([HDc + 1, 512], F32, tag="otile", name="po")
                    po = po_all[h]
                    al = max(0, 128 * i - qs0)
                    P = pbp.tile([128, PW], BF16, tag="ptile", name="P")
                    nc.scalar.activation(
                        P[:, ds(al, PW - al)],
                        S[:, ds(al, PW - al)],
                        mybir.ActivationFunctionType.Exp,
                        scale=float(cfg.scale),
                    )
                    nc.tensor.matmul(
                        po[:, ds(al, PW - al)],
                        Vb[:, i, ds((HDc + 1) * h, HDc + 1)],
                        P[:, ds(al, PW - al)],
                        start=(i == 0), stop=(i == ilast),
                        skip_group_check=(al > 0 or i != ilast),
                    )
                    if i == ilast:
                        # the copy exists to free the PSUM accumulator for the
                        # next head; the last head of the last pass normalizes
                        # straight from PSUM (shorter end-of-kernel chain)
                        if p == NPASS - 1 and h == HLOCc - 1:
                            osrc = po
                        else:
                            osrc = nrm.tile([HDc + 1, 512], F32, tag="osnap", name="osnap")
                            nc.vector.tensor_copy(osrc, po)
                        rec = nrm.tile([1, 512], F32, tag="rec", name="rec")
                        nc.vector.reciprocal(rec, osrc[HDc : HDc + 1, :])
                        rb = nrm.tile([64, 512], F32, tag="rb", name="rb")
                        nc.gpsimd.partition_broadcast(rb, rec)
                        nc.vector.tensor_mul(
                            OTs[64 * hp : 64 * hp + 64, hf, ds(qs0, 512)],
                            osrc[0:HDc, :],
                            rb,
                        )
                if p == NPASS - 1:
                    OFB = emit_ag_half(p, 1)
                    emit_outproj_split(p, OFA, OFB)
                else:
                    OF_prev = emit_ag(p)
        psum2_cm.__exit__(None, None, None)
        psum_cm.__exit__(None, None, None)


def make_program(cfg=None, groups=None, unroll=1):
    cfg = cfg or Cfg()
    groups = groups or REPLICA_GROUPS
    nc = bacc.Bacc("TRN2", target_bir_lowering=False, debug=False, num_devices=NCORES)
    x = nc.dram_tensor("x", [cfg.DM, cfg.L], F32, kind="ExternalInput").ap()
    wqkv = nc.dram_tensor("wqkv", [cfg.DM, 3 * cfg.FLOC], F32, kind="ExternalInput").ap()
    bq = nc.dram_tensor("bq", [cfg.FLOC], F32, kind="ExternalInput").ap()
    bk = nc.dram_tensor("bk", [cfg.FLOC], F32, kind="ExternalInput").ap()
    bv = nc.dram_tensor("bv", [cfg.FLOC], F32, kind="ExternalInput").ap()
    wo = nc.dram_tensor("wo", [cfg.DM, cfg.FLOC], F32, kind="ExternalInput").ap()
    bo = nc.dram_tensor("bo", [cfg.FLOC], F32, kind="ExternalInput").ap()
    out = nc.dram_tensor("out", [cfg.L, cfg.FLOC], F32, kind="ExternalOutput").ap()
    with tile.TileContext(nc) as tc:
        nc.tc = tc
        for _ in range(unroll):
            build_body(nc, cfg, x, wqkv, bq, bk, bv, wo, bo, out, groups)
    nc.compile()
    return nc


def shard_inputs(x, w_qkv, b_qkv, w_out, b_out, cfg=None):
    """Full inputs -> list of 8 per-core input dicts (x pre-transposed)."""
    cfg = cfg or Cfg()
    FL = cfg.FLOC
    DMF = cfg.DM
    in_maps = []
    xt = [np.ascontiguousarray(x[b].T) for b in range(BS)]
    for c in range(NCORES):
        b, r = divmod(c, GRP)
        q0 = r * FL
        in_maps.append({
            "x": xt[b],
            "wqkv": np.ascontiguousarray(np.concatenate([
                w_qkv[:, q0 : q0 + FL],
                w_qkv[:, DMF + q0 : DMF + q0 + FL],
                w_qkv[:, 2 * DMF + q0 : 2 * DMF + q0 + FL],
            ], axis=1)),
            "bq": np.ascontiguousarray(b_qkv[q0 : q0 + FL]),
            "bk": np.ascontiguousarray(b_qkv[DMF + q0 : DMF + q0 + FL]),
            "bv": np.ascontiguousarray(b_qkv[2 * DMF + q0 : 2 * DMF + q0 + FL]),
            "wo": np.ascontiguousarray(w_out[:, q0 : q0 + FL]),
            "bo": np.ascontiguousarray(b_out[q0 : q0 + FL]),
        })
    return in_maps


def gather_output(results, cfg=None):
    cfg = cfg or Cfg()
    FL = cfg.FLOC
    out = np.empty((BS, cfg.L, cfg.DM), np.float32)
    for c in range(NCORES):
        b, r = divmod(c, GRP)
        out[b, :, r * FL : (r + 1) * FL] = results[c]["out"]
    return out


_PROGRAM = None


def _get_program():
    global _PROGRAM
    if _PROGRAM is None:
        _PROGRAM = make_program()
    return _PROGRAM


def kernel(x, w_qkv, b_qkv, w_out, b_out):
    x = np.asarray(x, np.float32)
    w_qkv = np.asarray(w_qkv, np.float32)
    b_qkv = np.asarray(b_qkv, np.float32)
    w_out = np.asarray(w_out, np.float32)
    b_out = np.asarray(b_out, np.float32)
    nc = _get_program()
    in_maps = shard_inputs(x, w_qkv, b_qkv, w_out, b_out)
    res = run_bass_kernel_spmd(nc, in_maps, list(range(NCORES)))
    return gather_output(res.results)


# revision 45
# speedup vs baseline: 1.2195x; 1.1824x over previous
"""Trainium2 Bass kernel for causal multi-head attention (dense transformer block).

Problem: x[2,2048,1024] -> qkv proj -> 16-head causal attention (scale 1/sqrt(1024))
         -> out proj.  8 NeuronCores.

Sharding: core c handles batch b=c//4 and head-group r=c%4 (heads 4r..4r+3).
  - qkv weights column-sharded by head group (q/k/v slices of 256 cols each);
    x is passed to each core pre-transposed (dm-major) so the kernel stages
    x^T with plain cast-DMAs and no on-device transposes.
  - attention computed fully on-core in a transposed layout:
      S^T[k,q] = K^T-chunk (stationary) x Q^T (moving) on the PE.
      Q^T/K^T are stored in fp8-e4m3 DoubleRow layout (head dim split 32+32
      across the packed plane; weight columns host-permuted to match) so each
      scores matmul runs at 0.5 cycles/column. V and all d_model-contraction
      GEMMs stay bf16 (fp8 there fails the 2e-2 gate).
      P = exp(S/32); causal masking is done pre-exp by accumulating a
      -1e9 strict-upper-triangular matrix into the diagonal S block on the
      PE (mask matmul against an identity moving tile), so exp produces
      exact zeros below the diagonal and no post-exp masking is needed.
      The denominator comes from appending a ones-column to V so that
      O^T = [V|1]^T P gives sums in the last row.
  - q is processed in four 512-column passes. The attention job stream
    (scores three jobs ahead of the attnV that parks on its exp) is the
    backbone; qkv projection matmuls for the next token block are emitted
    between jobs in ~2-matmul units, and all deferred out-projections ride
    inside the exp-bound final pass, so the PE stays fed while the ACT
    engine works through the exp wall.
  - attnV matmuls stream only the causally-valid P columns (al-trimmed
    partial-region PSUM accumulation); PSUM->SBUF copies ride the DVE
    (tensor_scalar_add applies the per-partition qk bias), keeping ACT free
    for exps.
  - AllGather (bf16, groups of 4 cores sharing a batch) assembles all heads'
    outputs feature-major per 512-token block; the final pass gathers its
    two feature halves separately so the first half overlaps the remaining
    attention jobs. Out-proj is column-sharded against the gathered feature
    dim; biases are applied via rank-1 (K=1) matmul accumulation.

kernel(**inputs) takes the FULL fp32 inputs and returns the FULL output.
"""

import sys

sys.path.insert(0, "/opt/trn_rl_repo")

import numpy as np

import concourse.bass as bass
import concourse.bacc as bacc
import concourse.mybir as mybir
import concourse.tile as tile
from concourse.bass import ds, ts
from concourse.bass_utils import run_bass_kernel_spmd
from concourse.masks import make_upper_triangular

F32 = mybir.dt.float32
BF16 = mybir.dt.bfloat16
FP8 = mybir.dt.float8e4

# ---------------------------------------------------------------- dims
BS, L, DM, H = 2, 2048, 1024, 16
HD = 64                      # head dim
NCORES = 8
GRP = 4                      # cores per batch group (head-parallel)
HLOC = H // GRP              # heads per core = 4
FLOC = HLOC * HD             # local features = 256
SCALE = 1.0 / float(np.sqrt(DM))
REPLICA_GROUPS = [[0, 1, 2, 3], [4, 5, 6, 7]]


class Cfg:
    """Geometry (parametrized so a small config can be tested quickly)."""

    def __init__(self, L=L, DM=DM, hloc=HLOC, hd=HD, npass=4):
        self.L, self.DM, self.HLOC, self.HD, self.NPASS = L, DM, hloc, hd, npass
        self.FLOC = hloc * hd
        self.NT = L // 128           # 128-token tiles
        self.NB = L // 512           # 512-token blocks
        self.NDM = DM // 128         # dmodel chunks
        self.PW = L // npass         # pass width (q columns per pass)
        self.NFT = self.FLOC // 128  # feature tiles for Q^T/K^T (2)
        self.scale = 1.0 / float(np.sqrt(DM))
        assert self.PW == 512 and self.FLOC % 128 == 0


def build_body(nc, cfg, x, wqkv, bq, bk, bv, wo, bo, out, groups):
    """Emit the per-core program (Tile framework) for one iteration.

    `x` here is the PRE-TRANSPOSED input: [DM, L] fp32 (dm-major).
    """
    NT, NB, NDM, PW, NFT = cfg.NT, cfg.NB, cfg.NDM, cfg.PW, cfg.NFT
    HLOCc, HDc, FLOCc = cfg.HLOC, cfg.HD, cfg.FLOC
    Lc, DMc = cfg.L, cfg.DM
    NPASS = cfg.NPASS
    tc = nc.tc

    with tc.tile_pool(name="const", bufs=1) as constp, \
         tc.tile_pool(name="persist", bufs=1) as pp:
        # ---------------- persistent SBUF tensors
        xT = pp.tile([128, NDM, Lc], BF16)                 # x^T  (dm-major)
        wqkvb = pp.tile([128, NDM, 3 * FLOCc], BF16)       # [wq|wk|wv] packed
        wqb = wqkvb[:, :, 0:FLOCc]
        wkb = wqkvb[:, :, FLOCc : 2 * FLOCc]
        wvb = wqkvb[:, :, 2 * FLOCc : 3 * FLOCc]
        wob = pp.tile([128, NDM, FLOCc], BF16)
        # Q^T/K^T in fp8-e4m3 DoubleRow layout: partition block [32h,32h+32)
        # = head h; plane 0 = head dims 0..31, plane 1 = dims 32..63 (weight
        # columns are host-permuted so the qkv PSUM comes out in this order)
        QT = pp.tile([128, 2, Lc], FP8)
        KT = pp.tile([128, 2, Lc], FP8)
        Vb = pp.tile([128, NT, HLOCc * (HDc + 1)], BF16)   # [V | ones] per token tile
        OTs = pp.tile([128, NFT, Lc], BF16)                # attention out^T (feature-major)

        # ---------------- PSUM pools for the whole kernel
        # bank budget: stile [128,512]x4 = 4 (pool S), otile [65,512]x2 +
        # work [128,512]x2 = 4 (pool W) -> 8 banks.
        # Opened before staging, so no pool-boundary barrier ever lands on
        # the critical path.
        psum_cm = tc.tile_pool(name="psumS", bufs=4, space="PSUM")
        psum = psum_cm.__enter__()
        psum2_cm = tc.tile_pool(name="psumW", bufs=2, space="PSUM")
        psum2 = psum2_cm.__enter__()

        # PE warmup: a few junk matmuls at the head so the p-state ramp
        # happens on dead time (the DMA-bound front), not on the first real
        # matmuls.
        NWARM = 17
        wsrc_t = pp.tile([128, 512], BF16, name="wsrc_t")
        nc.vector.memset(wsrc_t, 0.25)
        wps = psum.tile([128, 512], F32, tag="stile", name="wps")
        for r in range(NWARM):
            nc.tensor.matmul(wps, wsrc_t[:, 0:128], wsrc_t,
                             start=(r == 0), stop=(r == NWARM - 1))
        wout_t = pp.tile([128, 512], F32, name="wout_t")
        nc.vector.tensor_copy(wout_t, wps)

        # ---------------- constants
        maskM = constp.tile([128, 128], BF16)   # strict-upper -1e9 (q-row, k-col)
        ident = constp.tile([128, 128], BF16)   # identity (mask matmul moving tile)
        ones_r = constp.tile([1, 128], BF16)
        negrow = constp.tile([1, 128], BF16)    # -1e9 row (rank-1 strip mask)
        bq_f = constp.tile([128, NFT], F32)
        bk_f = constp.tile([128, NFT], F32)
        bvb = constp.tile([1, FLOCc], BF16)
        bob = constp.tile([1, FLOCc], BF16)

        def emit_consts_sync():
            # biases go over HWDGE (f32) + tiny DVE casts -- keeps the serial
            # Pool SWDGE queue free for the big weight/x cast-loads
            nc.sync.dma_start(bq_f, bq.rearrange("(f p) -> p f", p=128))
            nc.sync.dma_start(bk_f, bk.rearrange("(f p) -> p f", p=128))
            bv_st = constp.tile([1, 2 * FLOCc], F32, name="bv_st")
            nc.sync.dma_start(bv_st[:, 0:FLOCc], bv.rearrange("(a b) -> a b", a=1))
            nc.sync.dma_start(bv_st[:, FLOCc : 2 * FLOCc], bo.rearrange("(a b) -> a b", a=1))
            nc.vector.memset(ones_r, 1.0)
            nc.vector.memset(negrow, -1e9)
            nc.vector.tensor_copy(bvb, bv_st[:, 0:FLOCc])
            nc.vector.tensor_copy(bob, bv_st[:, FLOCc : 2 * FLOCc])
            # ones columns of Vb
            nc.vector.memset(
                Vb.rearrange("p t (h u) -> p t h u", u=HDc + 1)[:, :, :, HDc : HDc + 1], 1.0
            )

        def emit_consts_pool():
            # gpsimd-built mask constants: emitted AFTER the staging cast-DMAs
            # are queued so they don't delay the Pool SWDGE path.
            make_upper_triangular(nc, maskM, val=-1e9, diag=False)
            # identity: ones on/above diag, then keep only at-or-below diag
            make_upper_triangular(nc, ident, val=1.0, diag=True)
            nc.gpsimd.affine_select(
                out=ident, in_=ident,
                compare_op=mybir.AluOpType.is_ge,
                fill=0.0, base=0,
                pattern=[[-1, 128]], channel_multiplier=1,
            )

        # ---------------- weight + x staging
        # x arrives pre-transposed [DM, L]: each 512-token block loads with a
        # single fp32->bf16 cast DMA straight into the chunked xT layout.
        # wqkv is loaded in two dm-chunk halves so the first qkv matmuls
        # (which consume dm-chunks in order) start as soon as x-block0 and
        # the first half have landed.
        xTv = x.rearrange("(c p) t -> p c t", p=128)
        wv_ = wqkv.rearrange("(c p) f -> p c f", p=128)

        def stage_xblock(b):
            nc.gpsimd.dma_start(xT[:, :, ts(b, 512)], xTv[:, :, ts(b, 512)])

        nc.gpsimd.dma_start(wqkvb[:, 0 : NDM // 2, :], wv_[:, 0 : NDM // 2, :])
        stage_xblock(0)
        emit_consts_sync()
        nc.gpsimd.dma_start(wqkvb[:, NDM // 2 : NDM, :], wv_[:, NDM // 2 : NDM, :])
        for b in range(1, NB):
            stage_xblock(b)
        nc.gpsimd.dma_start(wob, wo.rearrange("(c p) f -> p c f", p=128))
        emit_consts_pool()

        # ---------------- qkv projection for one 512-token block, split into
        # 8 independently-emittable pieces so they can interleave into the
        # attention job stream (filling PE idle while ACT runs exps).
        # PSUM->SBUF copies ride the DVE (tensor_scalar_add applies the
        # per-partition bias), keeping the ACT engine free for exps.
        def qk_units(tb, pl, wsrc, dest, bias):
            st = {}
            def u_mm(c2):
                def f():
                    if c2 == 0:
                        st["t"] = psum2.tile([128, 512], F32, tag="work", name="qs")
                    for c in range(2 * c2, 2 * c2 + 2):
                        nc.tensor.matmul(
                            st["t"], wsrc[:, c, ts(pl, 128)], xT[:, c, ts(tb, 512)],
                            start=(c == 0), stop=(c == NDM - 1),
                        )
                    if c2 == NDM // 2 - 1:
                        nc.vector.tensor_scalar_add(dest[:, pl, ts(tb, 512)],
                                                    st["t"], bias[:, pl : pl + 1])
                return f
            return [u_mm(c2) for c2 in range(NDM // 2)]

        def v_units(tt):
            st = {}
            def u_mm(cg):
                def f():
                    if cg == 0:
                        st["pf"] = psum2.tile([128, 512], F32, tag="work",
                                              name="psv_full")
                    psv = st["pf"][:, 0:FLOCc]
                    for c in range(2 * cg, 2 * cg + 2):
                        nc.tensor.matmul(
                            psv, xT[:, c, ts(tt, 128)], wvb[:, c, :],
                            start=(c == 0), stop=False,
                        )
                    if cg == NDM // 2 - 1:
                        nc.tensor.matmul(psv, ones_r, bvb, start=False, stop=True)
                        nc.vector.tensor_copy(
                            Vb[:, tt, :].rearrange("p (h u) -> p h u", u=HDc + 1)[:, :, 0:HDc],
                            psv.rearrange("p (h d) -> p h d", d=HDc),
                        )
                return f
            return [u_mm(cg) for cg in range(NDM // 2)]

        def qkv_units(tb):
            units = []
            for pl in range(2):
                for (w, d, b) in ((wqb, QT, bq_f), (wkb, KT, bk_f)):
                    units.extend(qk_units(tb, pl, w, d, b))
            for tt in range(tb * 4, tb * 4 + 4):
                units.extend(v_units(tt))
            return units

        def emit_qkv_qk(tb):
            for pl in range(2):
                for (w, d, b) in ((wqb, QT, bq_f), (wkb, KT, bk_f)):
                    for f in qk_units(tb, pl, w, d, b):
                        f()

        # ---------------- attention + allgather + out projection
        with tc.tile_pool(name="pbuf", bufs=6) as pbp, \
             tc.tile_pool(name="nrm", bufs=6) as nrm, \
             tc.tile_pool(name="of", bufs=3) as ofp, \
             tc.tile_pool(name="osb", bufs=3) as osbp, \
             tc.tile_pool(name="dram", bufs=2, space="DRAM") as dramp:

            # ---- allgather + out-projection for one 512-token pass.
            # Emission is deferred into the NEXT pass's job stream: PE.SEQ is
            # in-order, and out-proj parked on the AllGather would otherwise
            # stall the next pass's scores.
            def emit_ag(p):
                """AllGather of pass p's OTs block + gathered-feature loads.
                Emitted right after the pass's jobs: everything here rides the
                DMA/Pool queues, so it cannot park the in-order PE queue."""
                q0 = p * PW
                ag_in = dramp.tile([NFT * 128, 512], BF16, tag="agin", name="ag_in")
                # NOTE: Shared-output collectives need >4 cores/group; with
                # 4-core groups the output must be a Local scratch tensor.
                ag_out = dramp.tile([GRP * NFT * 128, 512], BF16, tag="agout", name="ag_out")
                for t in range(NFT):
                    nc.sync.dma_start(ag_in[ts(t, 128), :], OTs[:, t, ds(q0, 512)])
                nc.gpsimd.collective_compute(
                    "AllGather",
                    mybir.AluOpType.bypass,
                    ins=[ag_in.opt()],
                    outs=[ag_out.opt()],
                    replica_groups=groups,
                )
                OF = ofp.tile([128, NDM, 512], BF16, tag="of", name="OF")
                # paired per-chunk loads at half the HWDGE slots
                agv = ag_out.rearrange("(c p) q -> p c q", p=128)
                for c2 in range(NDM // 2):
                    nc.sync.dma_start(OF[:, 2 * c2 : 2 * c2 + 2, :],
                                      agv[:, 2 * c2 : 2 * c2 + 2, :])
                return OF

            def outproj_units(p, OF):
                q0 = p * PW
                st = {}
                outv = out[ds(q0, 512), :].rearrange("(t p) f -> p t f", p=128)
                def u_ttl(ttl):
                    def f():
                        if ttl == 0:
                            st["osb"] = osbp.tile([128, 4, FLOCc], F32,
                                                  tag="osb", name="osb")
                        pout_full = psum2.tile([128, 512], F32, tag="work",
                                               name="pout_full")
                        pout = pout_full[:, 0:FLOCc]
                        for c in range(NDM):
                            nc.tensor.matmul(
                                pout, OF[:, c, ts(ttl, 128)], wob[:, c, :],
                                start=(c == 0), stop=False,
                            )
                        nc.tensor.matmul(pout, ones_r, bob, start=False, stop=True)
                        nc.vector.tensor_copy(st["osb"][:, ttl, :], pout)
                        if ttl == 1:
                            nc.sync.dma_start(outv[:, 0:2, :], st["osb"][:, 0:2, :])
                        elif ttl == 3:
                            nc.sync.dma_start(outv[:, 2:4, :], st["osb"][:, 2:4, :])
                    return f
                return [u_ttl(t) for t in range(4)]

            def emit_outproj(p, OF):
                for f in outproj_units(p, OF):
                    f()

            # ---- half-feature AllGather for the FINAL pass: heads {0,1}
            # (= OTs feature tile 0) gather while heads {2,3} still compute,
            # so only the second half-gather sits on the kernel tail.
            # Gathered chunk r of half ft maps to wob chunk 2r+ft.
            def emit_ag_half(p, fth):
                q0 = p * PW
                ag_in_h = dramp.tile([128, 512], BF16, tag="aginh", name="ag_in_h")
                ag_out_h = dramp.tile([GRP * 128, 512], BF16, tag="agouth", name="ag_out_h")
                nc.sync.dma_start(ag_in_h, OTs[:, fth, ds(q0, 512)])
                nc.gpsimd.collective_compute(
                    "AllGather",
                    mybir.AluOpType.bypass,
                    ins=[ag_in_h.opt()],
                    outs=[ag_out_h.opt()],
                    replica_groups=groups,
                )
                OFh = ofp.tile([128, GRP, 512], BF16, tag="ofh", name="OFh")
                agvh = ag_out_h.rearrange("(c p) q -> p c q", p=128)
                for c2 in range(GRP // 2):
                    nc.sync.dma_start(OFh[:, 2 * c2 : 2 * c2 + 2, :],
                                      agvh[:, 2 * c2 : 2 * c2 + 2, :])
                return OFh

            def emit_outproj_split(p, OFA, OFB):
                q0 = p * PW
                osb = osbp.tile([128, 4, FLOCc], F32, tag="osb", name="osb")
                outv = out[ds(q0, 512), :].rearrange("(t p) f -> p t f", p=128)
                for ttl in range(4):
                    pout_full = psum2.tile([128, 512], F32, tag="work", name="pout_full")
                    pout = pout_full[:, 0:FLOCc]
                    for r in range(GRP):
                        nc.tensor.matmul(
                            pout, OFA[:, r, ts(ttl, 128)], wob[:, 2 * r, :],
                            start=(r == 0), stop=False,
                        )
                    for r in range(GRP):
                        nc.tensor.matmul(
                            pout, OFB[:, r, ts(ttl, 128)], wob[:, 2 * r + 1, :],
                            start=False, stop=False,
                        )
                    nc.tensor.matmul(pout, ones_r, bob, start=False, stop=True)
                    nc.vector.tensor_copy(osb[:, ttl, :], pout)
                    nc.sync.dma_start(outv[:, ttl, :], osb[:, ttl, :])

            emit_qkv_qk(0)
            OF_all = {}
            op_units_all = {}
            for p in range(NPASS):
                ilast = 4 * p + 3
                qs0 = p * PW

                def emit_scores(h, i):
                    al = max(0, 128 * i - qs0)
                    is_diag = 128 * i >= qs0
                    S = psum.tile([128, 512], F32, tag="stile", name="S")
                    nc.tensor.matmul(
                        S[:, ds(al, 512 - al)],
                        KT[32 * h : 32 * h + 32, :, ts(i, 128)],
                        QT[32 * h : 32 * h + 32, :, ds(qs0 + al, 512 - al)],
                        start=True, stop=not is_diag,
                        skip_group_check=is_diag,
                        perf_mode=mybir.MatmulPerfMode.DoubleRow,
                        tile_position=(32 * h, 0),
                    )
                    if is_diag:
                        nc.tensor.matmul(
                            S[:, ds(al, 128)], maskM, ident,
                            start=False, stop=True,
                            skip_group_check=True,
                        )
                    return S

                # one flat (h, pair) stream with scores emitted one pair
                # ahead (PE.SEQ is in-order); qkv pieces for the NEXT token
                # block are interleaved between jobs to fill the PE while the
                # ACT engine works through the exps.
                jobs = [(h, i) for h in range(HLOCc) for i in range(ilast + 1)]
                units = qkv_units(p + 1) if p < NPASS - 1 else []
                nu = len(units)
                if p == 0:
                    # V units for block 0 woven in just ahead of the attnV
                    # that consumes them (job (h=0, i=tt) is at idx tt)
                    v0_at = {}
                    for tt in range(4):
                        v0_at[tt] = v_units(tt)
                # all deferred out-projections ride in the FINAL pass, which
                # is otherwise exp-bound with an idle PE
                op_at = {}
                if p == NPASS - 1:
                    op_at = {}
                    for k_ in range(3):
                        for t_ in range(4):
                            op_at[6 + 16 * k_ + 3 * t_] = (k_, t_)
                # just after heads {0,1} finish
                sp_half = 2 * (ilast + 1) + 1
                OFA = None
                po_all = {}
                Sq = [emit_scores(*jobs[k]) for k in range(min(3, len(jobs)))]
                for idx, (h, i) in enumerate(jobs):
                    hf, hp = h // 2, h % 2
                    if idx in op_at:
                        k_, t_ = op_at[idx]
                        if t_ == 0:
                            op_units = outproj_units(k_, OF_all[k_])
                            op_units_all[k_] = op_units
                        op_units_all[k_][t_]()
                    if p == NPASS - 1 and idx == sp_half:
                        OFA = emit_ag_half(p, 0)
                    S = Sq.pop(0)
                    if idx + 3 < len(jobs):
                        Sq.append(emit_scores(*jobs[idx + 3]))
                    if p == 0:
                        for f in v0_at.get(idx, ()):
                            f()
                    if nu:
                        for j in range(idx * nu // len(jobs),
                                       (idx + 1) * nu // len(jobs)):
                            units[j]()
                    if i == 0:
                        po_all[h] = psum2.tile([HDc + 1, 512], F32, tag="otile", name="po")
                    po = po_all[h]
                    al = max(0, 128 * i - qs0)
                    P = pbp.tile([128, 512], BF16, tag="ptile", name="P")
                    nc.scalar.activation(
                        P[:, ds(al, 512 - al)],
                        S[:, ds(al, 512 - al)],
                        mybir.ActivationFunctionType.Exp,
                        scale=float(cfg.scale),
                    )
                    nc.tensor.matmul(
                        po[:, ds(al, 512 - al)],
                        Vb[:, i, ds((HDc + 1) * h, HDc + 1)],
                        P[:, ds(al, 512 - al)],
                        start=(i == 0), stop=(i == ilast),
                        skip_group_check=True,
                    )
                    if i == ilast:
                        # the copy exists to free the PSUM accumulator for the
                        # next head; the last head of the last pass normalizes
                        # straight from PSUM (shorter end-of-kernel chain)
                        if p == NPASS - 1 and h == HLOCc - 1:
                            osrc = po
                        else:
                            osrc = nrm.tile([HDc + 1, 512], F32, tag="osnap", name="osnap")
                            nc.vector.tensor_copy(osrc, po)
                        rec = nrm.tile([1, 512], F32, tag="rec", name="rec")
                        nc.vector.reciprocal(rec, osrc[HDc : HDc + 1, :])
                        rb = nrm.tile([64, 512], F32, tag="rb", name="rb")
                        nc.gpsimd.partition_broadcast(rb, rec)
                        nc.vector.tensor_mul(
                            OTs[64 * hp : 64 * hp + 64, hf, ds(qs0, 512)],
                            osrc[0:HDc, :],
                            rb,
                        )
                if p == NPASS - 1:
                    OFB = emit_ag_half(p, 1)
                    emit_outproj_split(p, OFA, OFB)
                else:
                    OF_all[p] = emit_ag(p)
        psum2_cm.__exit__(None, None, None)
        psum_cm.__exit__(None, None, None)


def make_program(cfg=None, groups=None, unroll=1):
    cfg = cfg or Cfg()
    groups = groups or REPLICA_GROUPS
    nc = bacc.Bacc("TRN2", target_bir_lowering=False, debug=False, num_devices=NCORES)
    x = nc.dram_tensor("x", [cfg.DM, cfg.L], F32, kind="ExternalInput").ap()
    wqkv = nc.dram_tensor("wqkv", [cfg.DM, 3 * cfg.FLOC], F32, kind="ExternalInput").ap()
    bq = nc.dram_tensor("bq", [cfg.FLOC], F32, kind="ExternalInput").ap()
    bk = nc.dram_tensor("bk", [cfg.FLOC], F32, kind="ExternalInput").ap()
    bv = nc.dram_tensor("bv", [cfg.FLOC], F32, kind="ExternalInput").ap()
    wo = nc.dram_tensor("wo", [cfg.DM, cfg.FLOC], F32, kind="ExternalInput").ap()
    bo = nc.dram_tensor("bo", [cfg.FLOC], F32, kind="ExternalInput").ap()
    out = nc.dram_tensor("out", [cfg.L, cfg.FLOC], F32, kind="ExternalOutput").ap()
    with tile.TileContext(nc) as tc:
        nc.tc = tc
        for _ in range(unroll):
            build_body(nc, cfg, x, wqkv, bq, bk, bv, wo, bo, out, groups)
    nc.compile()
    return nc


def shard_inputs(x, w_qkv, b_qkv, w_out, b_out, cfg=None):
    """Full inputs -> list of 8 per-core input dicts (x pre-transposed)."""
    cfg = cfg or Cfg()
    FL = cfg.FLOC
    DMF = cfg.DM
    in_maps = []
    xt = [np.ascontiguousarray(x[b].T) for b in range(BS)]
    for c in range(NCORES):
        b, r = divmod(c, GRP)
        q0 = r * FL
        # permutation for the fp8 DoubleRow Q/K layout: new col j (j<128:
        # plane 0) = head (j//32), dim (j%32) + 32*plane
        perm = np.array([64 * ((j % 128) // 32) + 32 * (j // 128) + (j % 32)
                         for j in range(FL)])
        in_maps.append({
            "x": xt[b],
            "wqkv": np.ascontiguousarray(np.concatenate([
                w_qkv[:, q0 : q0 + FL][:, perm],
                w_qkv[:, DMF + q0 : DMF + q0 + FL][:, perm],
                w_qkv[:, 2 * DMF + q0 : 2 * DMF + q0 + FL],
            ], axis=1)),
            "bq": np.ascontiguousarray(b_qkv[q0 : q0 + FL][perm]),
            "bk": np.ascontiguousarray(b_qkv[DMF + q0 : DMF + q0 + FL][perm]),
            "bv": np.ascontiguousarray(b_qkv[2 * DMF + q0 : 2 * DMF + q0 + FL]),
            "wo": np.ascontiguousarray(w_out[:, q0 : q0 + FL]),
            "bo": np.ascontiguousarray(b_out[q0 : q0 + FL]),
        })
    return in_maps


def gather_output(results, cfg=None):
    cfg = cfg or Cfg()
    FL = cfg.FLOC
    out = np.empty((BS, cfg.L, cfg.DM), np.float32)
    for c in range(NCORES):
        b, r = divmod(c, GRP)
        out[b, :, r * FL : (r + 1) * FL] = results[c]["out"]
    return out


_PROGRAM = None


def _get_program():
    global _PROGRAM
    if _PROGRAM is None:
        _PROGRAM = make_program()
    return _PROGRAM


def kernel(x, w_qkv, b_qkv, w_out, b_out):
    x = np.asarray(x, np.float32)
    w_qkv = np.asarray(w_qkv, np.float32)
    b_qkv = np.asarray(b_qkv, np.float32)
    w_out = np.asarray(w_out, np.float32)
    b_out = np.asarray(b_out, np.float32)
    nc = _get_program()
    in_maps = shard_inputs(x, w_qkv, b_qkv, w_out, b_out)
    res = run_bass_kernel_spmd(nc, in_maps, list(range(NCORES)))
    return gather_output(res.results)


# revision 46
# speedup vs baseline: 1.2197x; 1.0002x over previous
"""Trainium2 Bass kernel for causal multi-head attention (dense transformer block).

Problem: x[2,2048,1024] -> qkv proj -> 16-head causal attention (scale 1/sqrt(1024))
         -> out proj.  8 NeuronCores.

Sharding: core c handles batch b=c//4 and head-group r=c%4 (heads 4r..4r+3).
  - qkv weights column-sharded by head group (q/k/v slices of 256 cols each);
    x is passed to each core pre-transposed (dm-major) so the kernel stages
    x^T with plain cast-DMAs and no on-device transposes.
  - attention computed fully on-core in a transposed layout:
      S^T[k,q] = K^T-chunk (stationary) x Q^T (moving) on the PE.
      Q^T/K^T are stored in fp8-e4m3 DoubleRow layout (head dim split 32+32
      across the packed plane; weight columns host-permuted to match) so each
      scores matmul runs at 0.5 cycles/column. V and all d_model-contraction
      GEMMs stay bf16 (fp8 there fails the 2e-2 gate).
      P = exp(S/32); causal masking is done pre-exp by accumulating a
      -1e9 strict-upper-triangular matrix into the diagonal S block on the
      PE (mask matmul against an identity moving tile), so exp produces
      exact zeros below the diagonal and no post-exp masking is needed.
      The denominator comes from appending a ones-column to V so that
      O^T = [V|1]^T P gives sums in the last row.
  - q is processed in four 512-column passes. The attention job stream
    (scores three jobs ahead of the attnV that parks on its exp) is the
    backbone; qkv projection matmuls for the next token block are emitted
    between jobs in ~2-matmul units, and all deferred out-projections ride
    inside the exp-bound final pass, so the PE stays fed while the ACT
    engine works through the exp wall.
  - attnV matmuls stream only the causally-valid P columns (al-trimmed
    partial-region PSUM accumulation); PSUM->SBUF copies ride the DVE
    (tensor_scalar_add applies the per-partition qk bias), keeping ACT free
    for exps.
  - AllGather (bf16, groups of 4 cores sharing a batch) assembles all heads'
    outputs feature-major per 512-token block; the final pass gathers its
    two feature halves separately so the first half overlaps the remaining
    attention jobs. Out-proj is column-sharded against the gathered feature
    dim; biases are applied via rank-1 (K=1) matmul accumulation.

kernel(**inputs) takes the FULL fp32 inputs and returns the FULL output.
"""

import sys

sys.path.insert(0, "/opt/trn_rl_repo")

import numpy as np

import concourse.bass as bass
import concourse.bacc as bacc
import concourse.mybir as mybir
import concourse.tile as tile
from concourse.bass import ds, ts
from concourse.bass_utils import run_bass_kernel_spmd
from concourse.masks import make_upper_triangular

F32 = mybir.dt.float32
BF16 = mybir.dt.bfloat16
FP8 = mybir.dt.float8e4

# ---------------------------------------------------------------- dims
BS, L, DM, H = 2, 2048, 1024, 16
HD = 64                      # head dim
NCORES = 8
GRP = 4                      # cores per batch group (head-parallel)
HLOC = H // GRP              # heads per core = 4
FLOC = HLOC * HD             # local features = 256
SCALE = 1.0 / float(np.sqrt(DM))
REPLICA_GROUPS = [[0, 1, 2, 3], [4, 5, 6, 7]]


class Cfg:
    """Geometry (parametrized so a small config can be tested quickly)."""

    def __init__(self, L=L, DM=DM, hloc=HLOC, hd=HD, npass=4):
        self.L, self.DM, self.HLOC, self.HD, self.NPASS = L, DM, hloc, hd, npass
        self.FLOC = hloc * hd
        self.NT = L // 128           # 128-token tiles
        self.NB = L // 512           # 512-token blocks
        self.NDM = DM // 128         # dmodel chunks
        self.PW = L // npass         # pass width (q columns per pass)
        self.NFT = self.FLOC // 128  # feature tiles for Q^T/K^T (2)
        self.scale = 1.0 / float(np.sqrt(DM))
        assert self.PW == 512 and self.FLOC % 128 == 0


def build_body(nc, cfg, x, wqkv, bq, bk, bv, wo, bo, out, groups):
    """Emit the per-core program (Tile framework) for one iteration.

    `x` here is the PRE-TRANSPOSED input: [DM, L] fp32 (dm-major).
    """
    NT, NB, NDM, PW, NFT = cfg.NT, cfg.NB, cfg.NDM, cfg.PW, cfg.NFT
    HLOCc, HDc, FLOCc = cfg.HLOC, cfg.HD, cfg.FLOC
    Lc, DMc = cfg.L, cfg.DM
    NPASS = cfg.NPASS
    tc = nc.tc

    with tc.tile_pool(name="const", bufs=1) as constp, \
         tc.tile_pool(name="persist", bufs=1) as pp:
        # ---------------- persistent SBUF tensors
        xT = pp.tile([128, NDM, Lc], BF16)                 # x^T  (dm-major)
        wqkvb = pp.tile([128, NDM, 3 * FLOCc], BF16)       # [wq|wk|wv] packed
        wqb = wqkvb[:, :, 0:FLOCc]
        wkb = wqkvb[:, :, FLOCc : 2 * FLOCc]
        wvb = wqkvb[:, :, 2 * FLOCc : 3 * FLOCc]
        wob = pp.tile([128, NDM, FLOCc], BF16)
        # Q^T/K^T in fp8-e4m3 DoubleRow layout: partition block [32h,32h+32)
        # = head h; plane 0 = head dims 0..31, plane 1 = dims 32..63 (weight
        # columns are host-permuted so the qkv PSUM comes out in this order)
        QT = pp.tile([128, 2, Lc], FP8)
        KT = pp.tile([128, 2, Lc], FP8)
        Vb = pp.tile([128, NT, HLOCc * (HDc + 1)], BF16)   # [V | ones] per token tile
        OTs = pp.tile([128, NFT, Lc], BF16)                # attention out^T (feature-major)

        # ---------------- PSUM pools for the whole kernel
        # bank budget: stile [128,512]x4 = 4 (pool S), otile [65,512]x2 +
        # work [128,512]x2 = 4 (pool W) -> 8 banks.
        # Opened before staging, so no pool-boundary barrier ever lands on
        # the critical path.
        psum_cm = tc.tile_pool(name="psumS", bufs=4, space="PSUM")
        psum = psum_cm.__enter__()
        psum2_cm = tc.tile_pool(name="psumW", bufs=2, space="PSUM")
        psum2 = psum2_cm.__enter__()

        # PE warmup: a few junk matmuls at the head so the p-state ramp
        # happens on dead time (the DMA-bound front), not on the first real
        # matmuls.
        NWARM = 12
        wsrc_t = pp.tile([128, 512], BF16, name="wsrc_t")
        nc.vector.memset(wsrc_t, 0.25)
        wps = psum.tile([128, 512], F32, tag="stile", name="wps")
        for r in range(NWARM):
            nc.tensor.matmul(wps, wsrc_t[:, 0:128], wsrc_t,
                             start=(r == 0), stop=(r == NWARM - 1))
        wout_t = pp.tile([128, 512], F32, name="wout_t")
        nc.vector.tensor_copy(wout_t, wps)

        # ---------------- constants
        maskM = constp.tile([128, 128], BF16)   # strict-upper -1e9 (q-row, k-col)
        ident = constp.tile([128, 128], BF16)   # identity (mask matmul moving tile)
        ones_r = constp.tile([1, 128], BF16)
        negrow = constp.tile([1, 128], BF16)    # -1e9 row (rank-1 strip mask)
        bq_f = constp.tile([128, NFT], F32)
        bk_f = constp.tile([128, NFT], F32)
        bvb = constp.tile([1, FLOCc], BF16)
        bob = constp.tile([1, FLOCc], BF16)

        def emit_consts_sync():
            # biases go over HWDGE (f32) + tiny DVE casts -- keeps the serial
            # Pool SWDGE queue free for the big weight/x cast-loads
            nc.sync.dma_start(bq_f, bq.rearrange("(f p) -> p f", p=128))
            nc.sync.dma_start(bk_f, bk.rearrange("(f p) -> p f", p=128))
            bv_st = constp.tile([1, 2 * FLOCc], F32, name="bv_st")
            nc.sync.dma_start(bv_st[:, 0:FLOCc], bv.rearrange("(a b) -> a b", a=1))
            nc.sync.dma_start(bv_st[:, FLOCc : 2 * FLOCc], bo.rearrange("(a b) -> a b", a=1))
            nc.vector.memset(ones_r, 1.0)
            nc.vector.memset(negrow, -1e9)
            nc.vector.tensor_copy(bvb, bv_st[:, 0:FLOCc])
            nc.vector.tensor_copy(bob, bv_st[:, FLOCc : 2 * FLOCc])
            # ones columns of Vb
            nc.vector.memset(
                Vb.rearrange("p t (h u) -> p t h u", u=HDc + 1)[:, :, :, HDc : HDc + 1], 1.0
            )

        def emit_consts_pool():
            # gpsimd-built mask constants: emitted AFTER the staging cast-DMAs
            # are queued so they don't delay the Pool SWDGE path.
            make_upper_triangular(nc, maskM, val=-1e9, diag=False)
            # identity: ones on/above diag, then keep only at-or-below diag
            make_upper_triangular(nc, ident, val=1.0, diag=True)
            nc.gpsimd.affine_select(
                out=ident, in_=ident,
                compare_op=mybir.AluOpType.is_ge,
                fill=0.0, base=0,
                pattern=[[-1, 128]], channel_multiplier=1,
            )

        # ---------------- weight + x staging
        # x arrives pre-transposed [DM, L]: each 512-token block loads with a
        # single fp32->bf16 cast DMA straight into the chunked xT layout.
        # wqkv is loaded in two dm-chunk halves so the first qkv matmuls
        # (which consume dm-chunks in order) start as soon as x-block0 and
        # the first half have landed.
        xTv = x.rearrange("(c p) t -> p c t", p=128)
        wv_ = wqkv.rearrange("(c p) f -> p c f", p=128)

        def stage_xblock(b):
            nc.gpsimd.dma_start(xT[:, :, ts(b, 512)], xTv[:, :, ts(b, 512)])

        nc.gpsimd.dma_start(wqkvb[:, 0 : NDM // 2, :], wv_[:, 0 : NDM // 2, :])
        stage_xblock(0)
        emit_consts_sync()
        nc.gpsimd.dma_start(wqkvb[:, NDM // 2 : NDM, :], wv_[:, NDM // 2 : NDM, :])
        for b in range(1, NB):
            stage_xblock(b)
        nc.gpsimd.dma_start(wob, wo.rearrange("(c p) f -> p c f", p=128))
        emit_consts_pool()

        # ---------------- qkv projection for one 512-token block, split into
        # 8 independently-emittable pieces so they can interleave into the
        # attention job stream (filling PE idle while ACT runs exps).
        # PSUM->SBUF copies ride the DVE (tensor_scalar_add applies the
        # per-partition bias), keeping the ACT engine free for exps.
        def qk_units(tb, pl, wsrc, dest, bias):
            st = {}
            def u_mm(c2):
                def f():
                    if c2 == 0:
                        st["t"] = psum2.tile([128, 512], F32, tag="work", name="qs")
                    for c in range(2 * c2, 2 * c2 + 2):
                        nc.tensor.matmul(
                            st["t"], wsrc[:, c, ts(pl, 128)], xT[:, c, ts(tb, 512)],
                            start=(c == 0), stop=(c == NDM - 1),
                        )
                    if c2 == NDM // 2 - 1:
                        nc.vector.tensor_scalar_add(dest[:, pl, ts(tb, 512)],
                                                    st["t"], bias[:, pl : pl + 1])
                return f
            return [u_mm(c2) for c2 in range(NDM // 2)]

        def v_units(tt):
            st = {}
            def u_mm(cg):
                def f():
                    if cg == 0:
                        st["pf"] = psum2.tile([128, 512], F32, tag="work",
                                              name="psv_full")
                    psv = st["pf"][:, 0:FLOCc]
                    for c in range(2 * cg, 2 * cg + 2):
                        nc.tensor.matmul(
                            psv, xT[:, c, ts(tt, 128)], wvb[:, c, :],
                            start=(c == 0), stop=False,
                        )
                    if cg == NDM // 2 - 1:
                        nc.tensor.matmul(psv, ones_r, bvb, start=False, stop=True)
                        nc.vector.tensor_copy(
                            Vb[:, tt, :].rearrange("p (h u) -> p h u", u=HDc + 1)[:, :, 0:HDc],
                            psv.rearrange("p (h d) -> p h d", d=HDc),
                        )
                return f
            return [u_mm(cg) for cg in range(NDM // 2)]

        def qkv_units(tb):
            units = []
            for pl in range(2):
                for (w, d, b) in ((wqb, QT, bq_f), (wkb, KT, bk_f)):
                    units.extend(qk_units(tb, pl, w, d, b))
            for tt in range(tb * 4, tb * 4 + 4):
                units.extend(v_units(tt))
            return units

        def emit_qkv_qk(tb):
            for pl in range(2):
                for (w, d, b) in ((wqb, QT, bq_f), (wkb, KT, bk_f)):
                    for f in qk_units(tb, pl, w, d, b):
                        f()

        # ---------------- attention + allgather + out projection
        with tc.tile_pool(name="pbuf", bufs=6) as pbp, \
             tc.tile_pool(name="nrm", bufs=6) as nrm, \
             tc.tile_pool(name="of", bufs=3) as ofp, \
             tc.tile_pool(name="osb", bufs=3) as osbp, \
             tc.tile_pool(name="dram", bufs=2, space="DRAM") as dramp:

            # ---- allgather + out-projection for one 512-token pass.
            # Emission is deferred into the NEXT pass's job stream: PE.SEQ is
            # in-order, and out-proj parked on the AllGather would otherwise
            # stall the next pass's scores.
            def emit_ag(p):
                """AllGather of pass p's OTs block + gathered-feature loads.
                Emitted right after the pass's jobs: everything here rides the
                DMA/Pool queues, so it cannot park the in-order PE queue."""
                q0 = p * PW
                ag_in = dramp.tile([NFT * 128, 512], BF16, tag="agin", name="ag_in")
                # NOTE: Shared-output collectives need >4 cores/group; with
                # 4-core groups the output must be a Local scratch tensor.
                ag_out = dramp.tile([GRP * NFT * 128, 512], BF16, tag="agout", name="ag_out")
                for t in range(NFT):
                    nc.sync.dma_start(ag_in[ts(t, 128), :], OTs[:, t, ds(q0, 512)])
                nc.gpsimd.collective_compute(
                    "AllGather",
                    mybir.AluOpType.bypass,
                    ins=[ag_in.opt()],
                    outs=[ag_out.opt()],
                    replica_groups=groups,
                )
                OF = ofp.tile([128, NDM, 512], BF16, tag="of", name="OF")
                # paired per-chunk loads at half the HWDGE slots
                agv = ag_out.rearrange("(c p) q -> p c q", p=128)
                for c2 in range(NDM // 2):
                    nc.sync.dma_start(OF[:, 2 * c2 : 2 * c2 + 2, :],
                                      agv[:, 2 * c2 : 2 * c2 + 2, :])
                return OF

            def outproj_units(p, OF):
                q0 = p * PW
                st = {}
                outv = out[ds(q0, 512), :].rearrange("(t p) f -> p t f", p=128)
                def u_ttl(ttl):
                    def f():
                        if ttl == 0:
                            st["osb"] = osbp.tile([128, 4, FLOCc], F32,
                                                  tag="osb", name="osb")
                        pout_full = psum2.tile([128, 512], F32, tag="work",
                                               name="pout_full")
                        pout = pout_full[:, 0:FLOCc]
                        for c in range(NDM):
                            nc.tensor.matmul(
                                pout, OF[:, c, ts(ttl, 128)], wob[:, c, :],
                                start=(c == 0), stop=False,
                            )
                        nc.tensor.matmul(pout, ones_r, bob, start=False, stop=True)
                        nc.vector.tensor_copy(st["osb"][:, ttl, :], pout)
                        if ttl == 1:
                            nc.sync.dma_start(outv[:, 0:2, :], st["osb"][:, 0:2, :])
                        elif ttl == 3:
                            nc.sync.dma_start(outv[:, 2:4, :], st["osb"][:, 2:4, :])
                    return f
                return [u_ttl(t) for t in range(4)]

            def emit_outproj(p, OF):
                for f in outproj_units(p, OF):
                    f()

            # ---- half-feature AllGather for the FINAL pass: heads {0,1}
            # (= OTs feature tile 0) gather while heads {2,3} still compute,
            # so only the second half-gather sits on the kernel tail.
            # Gathered chunk r of half ft maps to wob chunk 2r+ft.
            def emit_ag_half(p, fth):
                q0 = p * PW
                ag_in_h = dramp.tile([128, 512], BF16, tag="aginh", name="ag_in_h")
                ag_out_h = dramp.tile([GRP * 128, 512], BF16, tag="agouth", name="ag_out_h")
                nc.sync.dma_start(ag_in_h, OTs[:, fth, ds(q0, 512)])
                nc.gpsimd.collective_compute(
                    "AllGather",
                    mybir.AluOpType.bypass,
                    ins=[ag_in_h.opt()],
                    outs=[ag_out_h.opt()],
                    replica_groups=groups,
                )
                OFh = ofp.tile([128, GRP, 512], BF16, tag="ofh", name="OFh")
                agvh = ag_out_h.rearrange("(c p) q -> p c q", p=128)
                for c2 in range(GRP // 2):
                    nc.sync.dma_start(OFh[:, 2 * c2 : 2 * c2 + 2, :],
                                      agvh[:, 2 * c2 : 2 * c2 + 2, :])
                return OFh

            def emit_outproj_split(p, OFA, OFB):
                q0 = p * PW
                osb = osbp.tile([128, 4, FLOCc], F32, tag="osb", name="osb")
                outv = out[ds(q0, 512), :].rearrange("(t p) f -> p t f", p=128)
                for ttl in range(4):
                    pout_full = psum2.tile([128, 512], F32, tag="work", name="pout_full")
                    pout = pout_full[:, 0:FLOCc]
                    for r in range(GRP):
                        nc.tensor.matmul(
                            pout, OFA[:, r, ts(ttl, 128)], wob[:, 2 * r, :],
                            start=(r == 0), stop=False,
                        )
                    for r in range(GRP):
                        nc.tensor.matmul(
                            pout, OFB[:, r, ts(ttl, 128)], wob[:, 2 * r + 1, :],
                            start=False, stop=False,
                        )
                    nc.tensor.matmul(pout, ones_r, bob, start=False, stop=True)
                    nc.vector.tensor_copy(osb[:, ttl, :], pout)
                    nc.sync.dma_start(outv[:, ttl, :], osb[:, ttl, :])

            emit_qkv_qk(0)
            OF_all = {}
            op_units_all = {}
            for p in range(NPASS):
                ilast = 4 * p + 3
                qs0 = p * PW

                def emit_scores(h, i):
                    al = max(0, 128 * i - qs0)
                    is_diag = 128 * i >= qs0
                    S = psum.tile([128, 512], F32, tag="stile", name="S")
                    nc.tensor.matmul(
                        S[:, ds(al, 512 - al)],
                        KT[32 * h : 32 * h + 32, :, ts(i, 128)],
                        QT[32 * h : 32 * h + 32, :, ds(qs0 + al, 512 - al)],
                        start=True, stop=not is_diag,
                        skip_group_check=is_diag,
                        perf_mode=mybir.MatmulPerfMode.DoubleRow,
                        tile_position=(32 * h, 0),
                    )
                    if is_diag:
                        nc.tensor.matmul(
                            S[:, ds(al, 128)], maskM, ident,
                            start=False, stop=True,
                            skip_group_check=True,
                        )
                    return S

                # one flat (h, pair) stream with scores emitted one pair
                # ahead (PE.SEQ is in-order); qkv pieces for the NEXT token
                # block are interleaved between jobs to fill the PE while the
                # ACT engine works through the exps.
                jobs = [(h, i) for h in range(HLOCc) for i in range(ilast + 1)]
                units = qkv_units(p + 1) if p < NPASS - 1 else []
                nu = len(units)
                if p == 0:
                    # V units for block 0 woven in just ahead of the attnV
                    # that consumes them (job (h=0, i=tt) is at idx tt)
                    v0_at = {}
                    for tt in range(4):
                        v0_at[tt] = v_units(tt)
                # all deferred out-projections ride in the FINAL pass, which
                # is otherwise exp-bound with an idle PE
                op_at = {}
                if p == NPASS - 1:
                    op_at = {}
                    for k_ in range(3):
                        for t_ in range(4):
                            op_at[6 + 16 * k_ + 3 * t_] = (k_, t_)
                # just after heads {0,1} finish
                sp_half = 2 * (ilast + 1) + 1
                OFA = None
                po_all = {}
                Sq = [emit_scores(*jobs[k]) for k in range(min(3, len(jobs)))]
                for idx, (h, i) in enumerate(jobs):
                    hf, hp = h // 2, h % 2
                    if idx in op_at:
                        k_, t_ = op_at[idx]
                        if t_ == 0:
                            op_units = outproj_units(k_, OF_all[k_])
                            op_units_all[k_] = op_units
                        op_units_all[k_][t_]()
                    if p == NPASS - 1 and idx == sp_half:
                        OFA = emit_ag_half(p, 0)
                    S = Sq.pop(0)
                    if idx + 3 < len(jobs):
                        Sq.append(emit_scores(*jobs[idx + 3]))
                    if p == 0:
                        for f in v0_at.get(idx, ()):
                            f()
                    if nu:
                        for j in range(idx * nu // len(jobs),
                                       (idx + 1) * nu // len(jobs)):
                            units[j]()
                    if i == 0:
                        po_all[h] = psum2.tile([HDc + 1, 512], F32, tag="otile", name="po")
                    po = po_all[h]
                    al = max(0, 128 * i - qs0)
                    P = pbp.tile([128, 512], BF16, tag="ptile", name="P")
                    nc.scalar.activation(
                        P[:, ds(al, 512 - al)],
                        S[:, ds(al, 512 - al)],
                        mybir.ActivationFunctionType.Exp,
                        scale=float(cfg.scale),
                    )
                    nc.tensor.matmul(
                        po[:, ds(al, 512 - al)],
                        Vb[:, i, ds((HDc + 1) * h, HDc + 1)],
                        P[:, ds(al, 512 - al)],
                        start=(i == 0), stop=(i == ilast),
                        skip_group_check=True,
                    )
                    if i == ilast:
                        # the copy exists to free the PSUM accumulator for the
                        # next head; the last head of the last pass normalizes
                        # straight from PSUM (shorter end-of-kernel chain)
                        if p == NPASS - 1 and h == HLOCc - 1:
                            osrc = po
                        else:
                            osrc = nrm.tile([HDc + 1, 512], F32, tag="osnap", name="osnap")
                            nc.vector.tensor_copy(osrc, po)
                        rec = nrm.tile([1, 512], F32, tag="rec", name="rec")
                        nc.vector.reciprocal(rec, osrc[HDc : HDc + 1, :])
                        rb = nrm.tile([64, 512], F32, tag="rb", name="rb")
                        nc.gpsimd.partition_broadcast(rb, rec)
                        nc.vector.tensor_mul(
                            OTs[64 * hp : 64 * hp + 64, hf, ds(qs0, 512)],
                            osrc[0:HDc, :],
                            rb,
                        )
                if p == NPASS - 1:
                    OFB = emit_ag_half(p, 1)
                    emit_outproj_split(p, OFA, OFB)
                else:
                    OF_all[p] = emit_ag(p)
        psum2_cm.__exit__(None, None, None)
        psum_cm.__exit__(None, None, None)


def make_program(cfg=None, groups=None, unroll=1):
    cfg = cfg or Cfg()
    groups = groups or REPLICA_GROUPS
    nc = bacc.Bacc("TRN2", target_bir_lowering=False, debug=False, num_devices=NCORES)
    x = nc.dram_tensor("x", [cfg.DM, cfg.L], F32, kind="ExternalInput").ap()
    wqkv = nc.dram_tensor("wqkv", [cfg.DM, 3 * cfg.FLOC], F32, kind="ExternalInput").ap()
    bq = nc.dram_tensor("bq", [cfg.FLOC], F32, kind="ExternalInput").ap()
    bk = nc.dram_tensor("bk", [cfg.FLOC], F32, kind="ExternalInput").ap()
    bv = nc.dram_tensor("bv", [cfg.FLOC], F32, kind="ExternalInput").ap()
    wo = nc.dram_tensor("wo", [cfg.DM, cfg.FLOC], F32, kind="ExternalInput").ap()
    bo = nc.dram_tensor("bo", [cfg.FLOC], F32, kind="ExternalInput").ap()
    out = nc.dram_tensor("out", [cfg.L, cfg.FLOC], F32, kind="ExternalOutput").ap()
    with tile.TileContext(nc) as tc:
        nc.tc = tc
        for _ in range(unroll):
            build_body(nc, cfg, x, wqkv, bq, bk, bv, wo, bo, out, groups)
    nc.compile()
    return nc


def shard_inputs(x, w_qkv, b_qkv, w_out, b_out, cfg=None):
    """Full inputs -> list of 8 per-core input dicts (x pre-transposed)."""
    cfg = cfg or Cfg()
    FL = cfg.FLOC
    DMF = cfg.DM
    in_maps = []
    xt = [np.ascontiguousarray(x[b].T) for b in range(BS)]
    for c in range(NCORES):
        b, r = divmod(c, GRP)
        q0 = r * FL
        # permutation for the fp8 DoubleRow Q/K layout: new col j (j<128:
        # plane 0) = head (j//32), dim (j%32) + 32*plane
        perm = np.array([64 * ((j % 128) // 32) + 32 * (j // 128) + (j % 32)
                         for j in range(FL)])
        in_maps.append({
            "x": xt[b],
            "wqkv": np.ascontiguousarray(np.concatenate([
                w_qkv[:, q0 : q0 + FL][:, perm],
                w_qkv[:, DMF + q0 : DMF + q0 + FL][:, perm],
                w_qkv[:, 2 * DMF + q0 : 2 * DMF + q0 + FL],
            ], axis=1)),
            "bq": np.ascontiguousarray(b_qkv[q0 : q0 + FL][perm]),
            "bk": np.ascontiguousarray(b_qkv[DMF + q0 : DMF + q0 + FL][perm]),
            "bv": np.ascontiguousarray(b_qkv[2 * DMF + q0 : 2 * DMF + q0 + FL]),
            "wo": np.ascontiguousarray(w_out[:, q0 : q0 + FL]),
            "bo": np.ascontiguousarray(b_out[q0 : q0 + FL]),
        })
    return in_maps


def gather_output(results, cfg=None):
    cfg = cfg or Cfg()
    FL = cfg.FLOC
    out = np.empty((BS, cfg.L, cfg.DM), np.float32)
    for c in range(NCORES):
        b, r = divmod(c, GRP)
        out[b, :, r * FL : (r + 1) * FL] = results[c]["out"]
    return out


_PROGRAM = None


def _get_program():
    global _PROGRAM
    if _PROGRAM is None:
        _PROGRAM = make_program()
    return _PROGRAM


def kernel(x, w_qkv, b_qkv, w_out, b_out):
    x = np.asarray(x, np.float32)
    w_qkv = np.asarray(w_qkv, np.float32)
    b_qkv = np.asarray(b_qkv, np.float32)
    w_out = np.asarray(w_out, np.float32)
    b_out = np.asarray(b_out, np.float32)
    nc = _get_program()
    in_maps = shard_inputs(x, w_qkv, b_qkv, w_out, b_out)
    res = run_bass_kernel_spmd(nc, in_maps, list(range(NCORES)))
    return gather_output(res.results)
